# revision 8
# baseline (speedup 1.0000x reference)
"""DynamicA8W8 MoE FFN on 8 TRN2 NeuronCores.

Sizes (hardcoded from the problem spec):
  T=4096 tokens, H=4096 hidden, I=1408 intermediate, E=16 experts,
  equal contiguous groups of TPE=256 tokens per expert.

Sharding: expert-parallel == token-parallel here (contiguous equal groups).
Core c owns experts {2c, 2c+1} and tokens [512c, 512c+512). No cross-core
communication is needed; each core computes its own [512, H] output slab and
the host concatenates.

Per-core pipeline (v2):
  1. per-token dynamic quant of x in NATURAL layout (per-partition scale,
     no PE broadcast needed) -> int8 -> bf16, then DMA XBAR transpose
     (SBUF->SBUF, 2-byte) to the [h, t] layout mm1 needs. No PE transposes
     anywhere; x is loaded once (amax tiles are reused as quant source).
  2. grouped GEMM1 vs w13 (int8 weights DMA'd raw, cast to bf16 on chip;
     bf16 matmul is exact for int8 operands, fp32 PSUM accumulate).
     Expert 0's first column chunk runs one token-block at a time so the
     PE can start as soon as tb0's quant lands.
  3. SwiGLU epilogue fused with dequant scales (bf16 scale tables),
     dynamic requant to int8; hq transposed to [i, t] via DMA XBAR.
  4. GEMM2 vs w2, fused per-channel + per-token dequant, DMA out on the
     ACT HWDGE queue (loads own the SP queue).
  ~100 dependency-free warm-up matmuls run during the DMA lead-in to
  bring the PE HAM clock to 2.4 GHz before real matmuls start.
"""

import json

import numpy as np

import concourse.bass as bass
import concourse.bass2jax as bass2jax
import concourse.mybir as mybir
from concourse.bass_utils import run_bass_kernel_spmd
from concourse.masks import make_identity
from concourse.tile import TileContext

F32 = mybir.dt.float32
BF16 = mybir.dt.bfloat16
I8 = mybir.dt.int8
AF = mybir.ActivationFunctionType
ALU = mybir.AluOpType
AX = mybir.AxisListType

T, H, I, E = 4096, 4096, 1408, 16
NCORES = 8
E_LOC = E // NCORES            # 2 experts per core
TPE = T // E                   # 256 tokens per expert
T_LOC = E_LOC * TPE            # 512 tokens per core
NTB = T_LOC // 128             # 4 token blocks per core
KT1 = H // 128                 # 32 k-tiles for mm1
KT2 = I // 128                 # 11 k-tiles for mm2
NQ = 8                         # x row chunks (amax + quant granularity)
QW = H // NQ                   # 512
# gate/up column chunks (free dim of mm1, <=512 per PSUM bank).
# The small chunk goes first: expert 0 runs it split by token-block, so a
# smaller first chunk shortens the single-tb warmup passes.
I_CHUNKS = [(1024, 384), (0, 512), (512, 512)]
H_CHUNKS = [(c, 512) for c in range(0, H, 512)]
N_WARMUP = 100                 # HAM warm-up matmuls during DMA lead-in

# tunable buffer counts (sim-ablation knobs)
import os as _os
def _cfg(name, default):
    return int(_os.environ.get("K_" + name, default))
CFG_XT = _cfg("XT", 1)        # xt chunk bufs
CFG_XQBF = _cfg("XQBF", 2)
CFG_XQ8 = _cfg("XQ8", 2)
CFG_HQT = _cfg("HQT", 3)
CFG_OT = _cfg("OT", 2)
CFG_W2SC = _cfg("W2SC", 2)
CFG_HQI8 = _cfg("HQI8", 1)
CFG_W2I8 = _cfg("W2I8", 2)
CFG_WCAST = _cfg("WCAST", 6)
CFG_SPLIT0 = _cfg("SPLIT0", 1)  # split expert-0 chunk 0 by token block


# --- walrus workaround: this build rejects >1 sync wait per instruction.
# Split extras into standalone single-wait EventSemaphore instructions placed
# immediately before, on the same engine queue.
def _split_multi_waits(bir_json: bytes) -> bytes:
    j = json.loads(bir_json)
    changed = False
    for fn in j.get("functions", []):
        for blk in fn.get("blocks", []):
            out = []
            for inst in blk.get("instructions", []):
                si = inst.get("sync_info")
                waits = si.get("on_wait") if si else None
                if waits and len(waits) > 1:
                    spill, keep = waits[:-1], waits[-1:]
                    for k, w in enumerate(spill):
                        out.append({
                            "debug": inst.get("debug", 0),
                            "engine": inst["engine"],
                            "ins": [], "outs": [],
                            "name": f"{inst['name']}_w{k}",
                            "opcode": "EventSemaphore",
                            "sync_info": {"on_update": [], "on_wait": [w]},
                        })
                    si["on_wait"] = keep
                    changed = True
                out.append(inst)
            blk["instructions"] = out
    return json.dumps(j).encode() if changed else bir_json


_hook_installed = False


def _install_compile_hook():
    global _hook_installed
    if _hook_installed:
        return
    orig = bass2jax.compile_bir_kernel

    def wrapped(bir_json, tmpdir, neff_name="file.neff"):
        return orig(_split_multi_waits(bir_json), tmpdir, neff_name=neff_name)

    bass2jax.compile_bir_kernel = wrapped
    _hook_installed = True


def _cast_engine(nc, idx):
    """Round-robin the int8->bf16 weight casts across ACT and DVE.

    HW-measured int8->bf16 rates (ns per lane-elem): ACT 0.86, DVE 0.78,
    Pool 3.9 (one slow Pool cast on the critical path stalls 8 matmuls).
    """
    r = idx % 9
    if r < 5:
        return nc.scalar.copy
    return nc.vector.tensor_copy


def _build_program(reps=1):
    nc = bass.Bass()
    x_d = nc.declare_dram_parameter("x", [T_LOC, H], F32, isOutput=False)
    w13T_d = nc.declare_dram_parameter("w13T", [E_LOC, H, 2 * I], I8, isOutput=False)
    w2T_d = nc.declare_dram_parameter("w2T", [E_LOC, I, H], I8, isOutput=False)
    wsg_d = nc.declare_dram_parameter("wsg", [E_LOC, 128, I], F32, isOutput=False)
    wsu_d = nc.declare_dram_parameter("wsu", [E_LOC, 128, I], F32, isOutput=False)
    w2s_d = nc.declare_dram_parameter("w2s", [E_LOC, 128, H], BF16, isOutput=False)
    out_d = nc.declare_dram_parameter("out", [T_LOC, H], F32, isOutput=True)

    with TileContext(nc) as tc:
        with (
            tc.tile_pool(name="const", bufs=1) as const,
            tc.tile_pool(name="xload", bufs=2) as xload,
            tc.tile_pool(name="xq8", bufs=CFG_XQ8) as xq8p,
            tc.tile_pool(name="xqbf", bufs=CFG_XQBF) as xqbfp,
            tc.tile_pool(name="xqt", bufs=4) as xqtp,
            tc.tile_pool(name="small", bufs=4) as small,
            tc.tile_pool(name="wload", bufs=2) as wload,
            tc.tile_pool(name="wcast", bufs=CFG_WCAST) as wcast,
            tc.tile_pool(name="scales", bufs=2) as scalep,
            tc.tile_pool(name="hbuf", bufs=2) as hbuf,
            tc.tile_pool(name="hq", bufs=2) as hqp,
            tc.tile_pool(name="hqt", bufs=CFG_HQT) as hqtp,
            tc.tile_pool(name="outp", bufs=2) as outp,
            tc.tile_pool(name="pwarm", bufs=1, space="PSUM") as pwarm,
            tc.tile_pool(name="pg", bufs=2, space="PSUM") as pgp,
            tc.tile_pool(name="pu", bufs=2, space="PSUM") as pup,
            tc.tile_pool(name="p2", bufs=3, space="PSUM") as p2p,
        ):
            env = dict(locals())
            ident = const.tile([128, 128], BF16)
            make_identity(nc, ident)
            env["ident"] = ident
            for _rep in range(reps):
                if _rep > 0:
                    env["out_d"] = nc.dram_tensor(
                        f"out_rep{_rep}", [T_LOC, H], F32).ap()
                env["do_warmup"] = _rep == 0
                _emit_body(nc, tc, env)
    return nc


def _emit_body(nc, tc, pools):
    xload = pools["xload"]; xq8p = pools["xq8p"]; xqbfp = pools["xqbfp"]
    xqtp = pools["xqtp"]; small = pools["small"]
    wload = pools["wload"]; wcast = pools["wcast"]; scalep = pools["scalep"]
    hbuf = pools["hbuf"]; hqp = pools["hqp"]; hqtp = pools["hqtp"]
    outp = pools["outp"]; pgp = pools["pgp"]; pup = pools["pup"]
    p2p = pools["p2p"]; pwarm = pools["pwarm"]
    x_d = pools["x_d"]; w13T_d = pools["w13T_d"]; w2T_d = pools["w2T_d"]
    wsg_d = pools["wsg_d"]; wsu_d = pools["wsu_d"]; w2s_d = pools["w2s_d"]
    out_d = pools["out_d"]
    ident = pools["ident"]

    xqT = {}     # t-block -> [128h, KT1, 128t] bf16
    s1s = {}     # t-block -> [128, 1] f32 quant scale
    cast_n = [0]

    def cast(dst, src):
        _cast_engine(nc, cast_n[0])(dst, src)
        cast_n[0] += 1

    def warmup():
        # dependency-free matmuls to pull the PE HAM clock to 2.4 GHz
        # while the lead-in DMAs run
        pw = pwarm.tile([128, 128], F32, tag="pw")
        for i in range(N_WARMUP):
            nc.tensor.matmul(pw[:], ident[:], ident[:], start=True, stop=True)

    def quant_front(tb):
        # amax over the natural-layout row chunks (free-dim reduce);
        # keep the x tiles resident for the quant step
        xts = []
        am = None
        for hh in range(NQ):
            xt = xload.tile([128, QW], F32, tag=f"xt{hh}", name=f"xt{tb}_{hh}",
                            bufs=CFG_XT)
            nc.sync.dma_start(
                xt[:], x_d[tb * 128:(tb + 1) * 128, hh * QW:(hh + 1) * QW])
            xts.append(xt)
            amn = small.tile([128, 1], F32, tag="amax1", name=f"am{tb}_{hh}")
            nc.vector.tensor_reduce(amn[:], xt[:], axis=AX.X, op=ALU.max,
                                    apply_absolute_value=True)
            if hh > 0:
                am2 = small.tile([128, 1], F32, tag="amax1b",
                                 name=f"amc{tb}_{hh}")
                nc.vector.tensor_tensor(am2[:], am[:], amn[:], op=ALU.max)
                am = am2
            else:
                am = amn
        s1 = small.tile([128, 1], F32, tag="s1")
        nc.vector.tensor_scalar(s1[:], am[:], 1.0 / 127.0, None, op0=ALU.mult)
        inv1 = small.tile([128, 1], F32, tag="inv1")
        nc.vector.reciprocal(inv1[:], s1[:])
        s1s[tb] = s1
        return xts, inv1

    def quant_apply(tb, xts, inv1):
        # quantize in natural layout (per-partition scale), assemble the
        # bf16 row, then XBAR-transpose halves into the [h, t] layout
        xqbf = xqbfp.tile([128, H], BF16, tag="xqbf")
        for hh in range(NQ):
            xq8 = xq8p.tile([128, QW], I8, tag="xq8", name=f"xq8_{tb}_{hh}")
            nc.vector.tensor_scalar(xq8[:], xts[hh][:], inv1[:], None,
                                    op0=ALU.mult)
            (nc.scalar.copy if hh % 2 == 0 else nc.vector.tensor_copy)(
                xqbf[:, hh * QW:(hh + 1) * QW], xq8[:])
        xqt = xqtp.tile([128, KT1, 128], BF16, tag="xqT")
        half = KT1 // 2
        for j in range(2):
            nc.scalar.dma_start_transpose(
                xqt[:, j * half:(j + 1) * half, :],
                xqbf[:, j * (H // 2):(j + 1) * (H // 2)])
        xqT[tb] = xqt

    def mm1_loads(e, c0, cw):
        wg_i8 = [wload.tile([128, KT1 // 2, cw], I8, tag="wg_i8",
                            name=f"wg_i8_{e}_{c0}_{h2}") for h2 in range(2)]
        wu_i8 = [wload.tile([128, KT1 // 2, cw], I8, tag="wu_i8",
                            name=f"wu_i8_{e}_{c0}_{h2}") for h2 in range(2)]
        g_src = w13T_d[e, :, c0:c0 + cw].rearrange("(k p) o -> p k o", p=128)
        u_src = w13T_d[e, :, I + c0:I + c0 + cw].rearrange(
            "(k p) o -> p k o", p=128)
        for h2 in range(2):
            ksl = slice(h2 * (KT1 // 2), (h2 + 1) * (KT1 // 2))
            nc.sync.dma_start(wg_i8[h2][:], g_src[:, ksl, :])
            nc.sync.dma_start(wu_i8[h2][:], u_src[:, ksl, :])
        return wg_i8, wu_i8

    QK = 4  # k-tiles per cast op

    def cast_quad(w_i8, kq, cw, nm):
        h2, kkq = divmod(kq, (KT1 // 2) // QK)
        ks = slice(kkq * QK, (kkq + 1) * QK)
        w_bf = wcast.tile([128, QK, cw], BF16, tag="wbf", name=nm)
        cast(w_bf[:], w_i8[h2][:, ks, :])
        return w_bf

    def mm1_epilogue(e, i_tb, tb, c0, cw, pg, pu, wsg, wsu, hts, amaxes):
        gate = outp.tile([128, cw], F32, tag="gate")
        nc.vector.scalar_tensor_tensor(
            gate[:], pg, s1s[tb][:], wsg[:, c0:c0 + cw],
            op0=ALU.mult, op1=ALU.mult)
        up = outp.tile([128, cw], F32, tag="up")
        nc.vector.scalar_tensor_tensor(
            up[:], pu, s1s[tb][:], wsu[:, c0:c0 + cw],
            op0=ALU.mult, op1=ALU.mult)
        sg = outp.tile([128, cw], F32, tag="sg")
        nc.scalar.activation(sg[:], gate[:], AF.Silu)
        nc.vector.tensor_mul(hts[i_tb][:, c0:c0 + cw], sg[:], up[:])
        # per-chunk partial abs-max keeps the requant scale off the
        # critical path (ready right after the last chunk's h lands)
        prev = amaxes[i_tb]
        amp = small.tile([128, 1], F32, tag="amax2", name=f"am2p_{i_tb}_{c0}")
        nc.vector.tensor_reduce(amp[:], hts[i_tb][:, c0:c0 + cw],
                                axis=AX.X, op=ALU.max,
                                apply_absolute_value=True)
        if prev is not None:
            amn = small.tile([128, 1], F32, tag="amax2",
                             name=f"am2_{i_tb}_{c0}")
            nc.vector.tensor_tensor(amn[:], prev[:], amp[:], op=ALU.max)
            amaxes[i_tb] = amn
        else:
            amaxes[i_tb] = amp

    def mm1_chunk(e, tbs, c0, cw, wsg, wsu, hts, amaxes, loads):
        wg_i8, wu_i8 = loads
        pg = [pgp.tile([128, cw], F32, tag="pg", name=f"pg{i}")
              for i in range(len(tbs))]
        pu = [pup.tile([128, cw], F32, tag="pu", name=f"pu{i}")
              for i in range(len(tbs))]
        for kq in range(KT1 // QK):
            wg_bf = cast_quad(wg_i8, kq, cw, "wg_bf")
            wu_bf = cast_quad(wu_i8, kq, cw, "wu_bf")
            for dk in range(QK):
                k = kq * QK + dk
                st, sp = (k == 0), (k == KT1 - 1)
                for i_tb, tb in enumerate(tbs):
                    nc.tensor.matmul(pg[i_tb][:], xqT[tb][:, k, :],
                                     wg_bf[:, dk, :], start=st, stop=sp)
                    nc.tensor.matmul(pu[i_tb][:], xqT[tb][:, k, :],
                                     wu_bf[:, dk, :], start=st, stop=sp)
        for i_tb, tb in enumerate(tbs):
            mm1_epilogue(e, i_tb, tb, c0, cw, pg[i_tb][:], pu[i_tb][:],
                         wsg, wsu, hts, amaxes)

    def mm1_chunk_1tb(e, i_tb, tb, c0, cw, wsg, wsu, hts, amaxes, loads):
        # single token-block pass (separate casts) so the PE can start on
        # tb0 while tb1's quant is still in flight
        wg_i8, wu_i8 = loads
        pg = pgp.tile([128, cw], F32, tag="pg", name=f"pgs{i_tb}")
        pu = pup.tile([128, cw], F32, tag="pu", name=f"pus{i_tb}")
        for kq in range(KT1 // QK):
            wg_bf = cast_quad(wg_i8, kq, cw, "wg_bf")
            wu_bf = cast_quad(wu_i8, kq, cw, "wu_bf")
            for dk in range(QK):
                k = kq * QK + dk
                st, sp = (k == 0), (k == KT1 - 1)
                nc.tensor.matmul(pg[:], xqT[tb][:, k, :],
                                 wg_bf[:, dk, :], start=st, stop=sp)
                nc.tensor.matmul(pu[:], xqT[tb][:, k, :],
                                 wu_bf[:, dk, :], start=st, stop=sp)
        mm1_epilogue(e, i_tb, tb, c0, cw, pg[:], pu[:], wsg, wsu, hts,
                     amaxes)

    def requant_dve(ht, amax2):
        s2 = small.tile([128, 1], F32, tag="s2")
        nc.vector.tensor_scalar(s2[:], amax2[:], 1.0 / 127.0, None,
                                op0=ALU.mult)
        inv2 = small.tile([128, 1], F32, tag="inv2")
        nc.vector.reciprocal(inv2[:], s2[:])
        hq_i8 = hqp.tile([128, I], I8, tag="hq_i8", bufs=CFG_HQI8)
        hq_bf = hqp.tile([128, I], BF16, tag="hq_bf")
        for j, (a, b) in enumerate(((0, 512), (512, I))):
            nc.vector.tensor_scalar(hq_i8[:, a:b], ht[:, a:b], inv2[:], None,
                                    op0=ALU.mult)
            (nc.scalar.copy if j == 0 else nc.vector.tensor_copy)(
                hq_bf[:, a:b], hq_i8[:, a:b])
        return hq_bf, s2

    def requant_xpose(hq_bf):
        # hq [t, i] -> [i, t] via DMA XBAR (ACT HWDGE queue)
        hqt = hqtp.tile([128, KT2, 128], BF16, tag="hqT")
        nc.scalar.dma_start_transpose(hqt[:, :, :], hq_bf[:, :])
        return hqt

    def mm2_chunk(e, tbs, c0, cw, hqT_e, s2s, pending_stores):
        w2sc = outp.tile([128, cw], BF16, tag="w2sc", bufs=CFG_W2SC)
        nc.sync.dma_start(w2sc[:], w2s_d[e, :, c0:c0 + cw])
        w2_i8 = wload.tile([128, KT2, cw], I8, tag="w2_i8", bufs=CFG_W2I8)
        nc.sync.dma_start(
            w2_i8[:],
            w2T_d[e, :, c0:c0 + cw].rearrange("(k p) o -> p k o", p=128))
        p2 = [p2p.tile([128, cw], F32, tag="p2", name=f"p2_{i}")
              for i in range(len(tbs))]
        k = 0
        for qn in (4, 4, 3):
            w2_bf = wcast.tile([128, qn, cw], BF16, tag="wbf", name="w2_bf")
            cast(w2_bf[:], w2_i8[:, k:k + qn, :])
            for dk in range(qn):
                for i_tb in range(2):
                    nc.tensor.matmul(p2[i_tb][:], hqT_e[i_tb][:, k, :],
                                     w2_bf[:, dk, :], start=(k == 0),
                                     stop=(k == KT2 - 1))
                k += 1
        for i_tb, tb in enumerate(tbs):
            ot = outp.tile([128, cw], F32, tag="ot", bufs=CFG_OT)
            nc.vector.scalar_tensor_tensor(
                ot[:], p2[i_tb][:], s2s[i_tb][:], w2sc[:],
                op0=ALU.mult, op1=ALU.mult)
            pending_stores.append(
                (out_d[tb * 128:(tb + 1) * 128, c0:c0 + cw], ot))

    def flush_stores(pending_stores):
        for dst, ot in pending_stores:
            nc.scalar.dma_start(dst, ot[:])
        pending_stores.clear()

    # ---- Staged two-expert pipeline ----
    # Emission order == engine-queue order, so stages are interleaved to keep
    # the PE fed across quant/requant latency chains.
    assert E_LOC == 2
    loads_ = {}
    wsgs, wsus, htss, amaxs = {}, {}, {}, {}
    hqTs, s2ss = {}, {}

    def mm1_front(e):
        wsg = scalep.tile([128, I], F32, tag="wsg", name=f"wsg{e}")
        nc.sync.dma_start(wsg[:], wsg_d[e])
        wsu = scalep.tile([128, I], F32, tag="wsu", name=f"wsu{e}")
        nc.sync.dma_start(wsu[:], wsu_d[e])
        wsgs[e], wsus[e] = wsg, wsu
        htss[e] = [hbuf.tile([128, I], F32, tag="ht", name=f"ht{e}_{i}")
                   for i in range(2)]
        amaxs[e] = [None, None]

    def mm1_run_chunk(e, ci):
        tbs = [2 * e, 2 * e + 1]
        c0, cw = I_CHUNKS[ci]
        ld = loads_.pop((e, ci), None)
        if ld is None:
            ld = mm1_loads(e, c0, cw)
        mm1_chunk(e, tbs, c0, cw, wsgs[e], wsus[e], htss[e], amaxs[e], ld)

    def req_dve(e):
        hqbfs, s2ss[e] = [], []
        for i_tb in range(2):
            hq_bf, s2 = requant_dve(htss[e][i_tb], amaxs[e][i_tb])
            hqbfs.append(hq_bf)
            s2ss[e].append(s2)
        hqTs[e] = [requant_xpose(hq_bf) for hq_bf in hqbfs]

    # --- lead-in: tb0 amax stream first, then expert-0 chunk-0 weights ---
    if pools.get("do_warmup", True):
        warmup()
    xts0, inv_a = quant_front(0)
    loads_[(0, 0)] = mm1_loads(0, *I_CHUNKS[0])
    xts1, inv_b = quant_front(1)
    mm1_front(0)
    quant_apply(0, xts0, inv_a)
    c0, cw = I_CHUNKS[0]
    if CFG_SPLIT0:
        # chunk 0 of expert 0 split per token-block: PE starts on tb0 alone
        mm1_chunk_1tb(0, 0, 0, c0, cw, wsgs[0], wsus[0], htss[0], amaxs[0],
                      loads_[(0, 0)])
        quant_apply(1, xts1, inv_b)
        loads_[(0, 1)] = mm1_loads(0, *I_CHUNKS[1])
        mm1_chunk_1tb(0, 1, 1, c0, cw, wsgs[0], wsus[0], htss[0], amaxs[0],
                      loads_.pop((0, 0)))
    else:
        quant_apply(1, xts1, inv_b)
        mm1_run_chunk(0, 0)
        loads_[(0, 1)] = mm1_loads(0, *I_CHUNKS[1])
    # expert-1 token quant hides under expert-0 mm1
    xts2, inv_c = quant_front(2)
    mm1_run_chunk(0, 1)
    quant_apply(2, xts2, inv_c)
    xts3, inv_d = quant_front(3)
    mm1_front(1)
    mm1_run_chunk(0, 2)
    quant_apply(3, xts3, inv_d)
    # requant scale chain (DVE/ACT + XBAR) for e0; e1 matmuls keep PE busy
    req_dve(0)
    stores = []
    mm1_run_chunk(1, 0)
    for ci in range(5):
        mm2_chunk(0, [0, 1], *H_CHUNKS[ci], hqTs[0], s2ss[0], stores)
    mm1_run_chunk(1, 1)
    flush_stores(stores)
    mm1_run_chunk(1, 2)
    req_dve(1)
    # e0's mm2 tail covers e1's requant chain
    for ci in range(5, 8):
        mm2_chunk(0, [0, 1], *H_CHUNKS[ci], hqTs[0], s2ss[0], stores)
    flush_stores(stores)
    for ci in range(8):
        mm2_chunk(1, [2, 3], *H_CHUNKS[ci], hqTs[1], s2ss[1], stores)
        flush_stores(stores)


_cached_nc = None


def _make_in_maps(x, w13, w2, w13_scale, smooth_scale_2, w2_scale):
    import ml_dtypes
    x = np.asarray(x, dtype=np.float32)
    w13 = np.asarray(w13).astype(np.int8, copy=False)
    w2 = np.asarray(w2).astype(np.int8, copy=False)
    w13_scale = np.asarray(w13_scale, dtype=np.float32)
    smooth_scale_2 = np.asarray(smooth_scale_2, dtype=np.float32)
    w2_scale = np.asarray(w2_scale, dtype=np.float32)

    # Fold the (linear) smooth scale into the up-projection dequant scale.
    wsu_full = w13_scale[:, I:] * smooth_scale_2
    wsg_full = w13_scale[:, :I]
    w2s_full = w2_scale.astype(ml_dtypes.bfloat16)

    in_maps = []
    for c in range(NCORES):
        es = slice(E_LOC * c, E_LOC * (c + 1))
        ts = slice(T_LOC * c, T_LOC * (c + 1))
        in_maps.append({
            "x": np.ascontiguousarray(x[ts]),
            "w13T": np.ascontiguousarray(w13[es].transpose(0, 2, 1)),
            "w2T": np.ascontiguousarray(w2[es].transpose(0, 2, 1)),
            "wsg": np.ascontiguousarray(
                np.broadcast_to(wsg_full[es, None, :], (E_LOC, 128, I))),
            "wsu": np.ascontiguousarray(
                np.broadcast_to(wsu_full[es, None, :], (E_LOC, 128, I))),
            "w2s": np.ascontiguousarray(
                np.broadcast_to(w2s_full[es, None, :], (E_LOC, 128, H))),
        })
    return in_maps


def _run(in_maps, **kwargs):
    global _cached_nc
    _install_compile_hook()
    if _cached_nc is None:
        _cached_nc = _build_program()
    return run_bass_kernel_spmd(_cached_nc, in_maps, list(range(NCORES)),
                                **kwargs)


def kernel(x, w13, w2, w13_scale, smooth_scale_2, w2_scale, expert_tokens):
    # expert_tokens describes the fixed equal contiguous grouping (the
    # reference ignores it); we rely on that same grouping.
    del expert_tokens
    in_maps = _make_in_maps(x, w13, w2, w13_scale, smooth_scale_2, w2_scale)
    res = _run(in_maps)
    return np.concatenate([res.results[c]["out"] for c in range(NCORES)],
                          axis=0)


def run_profiled(x, w13, w2, w13_scale, smooth_scale_2, w2_scale,
                 expert_tokens):
    """test.py helper: run with NTFF profiling, return BassKernelResults."""
    del expert_tokens
    in_maps = _make_in_maps(x, w13, w2, w13_scale, smooth_scale_2, w2_scale)
    return _run(in_maps, trace=True)


# revision 9
# speedup vs baseline: 1.1174x; 1.1174x over previous
"""DynamicA8W8 MoE FFN on 8 TRN2 NeuronCores.

Sizes (hardcoded from the problem spec):
  T=4096 tokens, H=4096 hidden, I=1408 intermediate, E=16 experts,
  equal contiguous groups of TPE=256 tokens per expert.

Sharding: expert-parallel == token-parallel here (contiguous equal groups).
Core c owns experts {2c, 2c+1} and tokens [512c, 512c+512). No cross-core
communication is needed; each core computes its own [512, H] output slab and
the host concatenates.

Per-core pipeline:
  1. per-token dynamic quant of x -> int8 (RNE+saturate via f32->int8 copy),
     exact in bf16; PE-transpose to [h, t] layout for use as matmul stationary.
  2. grouped GEMM1 vs w13 (int8 weights DMA'd raw, cast to bf16 on chip;
     bf16 matmul is exact for int8 operands, fp32 PSUM accumulate).
  3. SwiGLU epilogue fused with dequant scales, dynamic requant to int8.
  4. GEMM2 vs w2, fused per-channel + per-token dequant, DMA out.
"""

import json

import numpy as np

import concourse.bass as bass
import concourse.bass2jax as bass2jax
import concourse.mybir as mybir
from concourse.bass_utils import run_bass_kernel_spmd
from concourse.masks import make_identity
from concourse.tile import TileContext

F32 = mybir.dt.float32
BF16 = mybir.dt.bfloat16
I8 = mybir.dt.int8
AF = mybir.ActivationFunctionType
ALU = mybir.AluOpType
AX = mybir.AxisListType

T, H, I, E = 4096, 4096, 1408, 16
NCORES = 8
E_LOC = E // NCORES            # 2 experts per core
TPE = T // E                   # 256 tokens per expert
T_LOC = E_LOC * TPE            # 512 tokens per core
NTB = T_LOC // 128             # 4 token blocks per core
KT1 = H // 128                 # 32 k-tiles for mm1
KT2 = I // 128                 # 11 k-tiles for mm2
# gate/up column chunks (free dim of mm1, <=512 per PSUM bank)
I_CHUNKS = [(0, 512), (512, 512), (1024, 384)]
H_CHUNKS = [(c, 512) for c in range(0, H, 512)]


# --- walrus workaround: this build rejects >1 sync wait per instruction.
# Split extras into standalone single-wait EventSemaphore instructions placed
# immediately before, on the same engine queue.
def _split_multi_waits(bir_json: bytes) -> bytes:
    j = json.loads(bir_json)
    changed = False
    for fn in j.get("functions", []):
        for blk in fn.get("blocks", []):
            out = []
            for inst in blk.get("instructions", []):
                si = inst.get("sync_info")
                waits = si.get("on_wait") if si else None
                if waits and len(waits) > 1:
                    spill, keep = waits[:-1], waits[-1:]
                    for k, w in enumerate(spill):
                        out.append({
                            "debug": inst.get("debug", 0),
                            "engine": inst["engine"],
                            "ins": [], "outs": [],
                            "name": f"{inst['name']}_w{k}",
                            "opcode": "EventSemaphore",
                            "sync_info": {"on_update": [], "on_wait": [w]},
                        })
                    si["on_wait"] = keep
                    changed = True
                out.append(inst)
            blk["instructions"] = out
    return json.dumps(j).encode() if changed else bir_json


_hook_installed = False


def _install_compile_hook():
    global _hook_installed
    if _hook_installed:
        return
    orig = bass2jax.compile_bir_kernel

    def wrapped(bir_json, tmpdir, neff_name="file.neff"):
        return orig(_split_multi_waits(bir_json), tmpdir, neff_name=neff_name)

    bass2jax.compile_bir_kernel = wrapped
    _hook_installed = True


def _cast_engine(nc, idx):
    """Round-robin the int8->bf16 weight casts across ACT/Pool/DVE.

    Balance for the engine rates (ACT 1.2G, DVE 0.96G, Pool ~0.72G effective)
    and each engine's other work: ACT 3/8, Pool 3/8, DVE 2/8.
    """
    # HW-measured int8->bf16 rates (ns per lane-elem): ACT 0.86, DVE 0.78,
    # Pool 3.9 (gpsimd is ~4x slower than the cost model thinks, and one slow
    # cast on the critical path stalls 8 matmuls) -- so no Pool casts at all.
    r = idx % 9
    if r < 5:
        return nc.scalar.copy
    return nc.vector.tensor_copy


def _build_program(reps=1):
    nc = bass.Bass()
    x_d = nc.declare_dram_parameter("x", [T_LOC, H], F32, isOutput=False)
    xT_d = nc.declare_dram_parameter("xT", [H, T_LOC], F32, isOutput=False)
    w13T_d = nc.declare_dram_parameter("w13T", [E_LOC, H, 2 * I], I8, isOutput=False)
    w2T_d = nc.declare_dram_parameter("w2T", [E_LOC, I, H], I8, isOutput=False)
    wsg_d = nc.declare_dram_parameter("wsg", [E_LOC, 128, I], F32, isOutput=False)
    wsu_d = nc.declare_dram_parameter("wsu", [E_LOC, 128, I], F32, isOutput=False)
    w2s_d = nc.declare_dram_parameter("w2s", [E_LOC, 128, H], F32, isOutput=False)
    out_d = nc.declare_dram_parameter("out", [T_LOC, H], F32, isOutput=True)

    with TileContext(nc) as tc:
        with (
            tc.tile_pool(name="const", bufs=1) as const,
            tc.tile_pool(name="xload", bufs=4) as xload,
            tc.tile_pool(name="xq", bufs=1) as xqp,
            tc.tile_pool(name="xqt", bufs=4) as xqtp,
            tc.tile_pool(name="small", bufs=4) as small,
            tc.tile_pool(name="wload", bufs=2) as wload,
            tc.tile_pool(name="wcast", bufs=7) as wcast,
            tc.tile_pool(name="scales", bufs=2) as scalep,
            tc.tile_pool(name="hbuf", bufs=2) as hbuf,
            tc.tile_pool(name="hq", bufs=2) as hqp,
            tc.tile_pool(name="outp", bufs=2) as outp,
            tc.tile_pool(name="pt", bufs=2, space="PSUM") as ptp,
            tc.tile_pool(name="pg", bufs=2, space="PSUM") as pgp,
            tc.tile_pool(name="pu", bufs=2, space="PSUM") as pup,
            tc.tile_pool(name="p2", bufs=2, space="PSUM") as p2p,
        ):
            env = dict(locals())
            ident = const.tile([128, 128], BF16)
            make_identity(nc, ident)
            env["ident"] = ident
            ident_f32 = const.tile([128, 128], F32)
            make_identity(nc, ident_f32)
            env["ident_f32"] = ident_f32
            ones_row = const.tile([128, 128], F32)
            nc.vector.memset(ones_row[:], 1.0)
            env["ones_row"] = ones_row
            for _rep in range(reps):
                if _rep > 0:
                    env["out_d"] = nc.dram_tensor(
                        f"out_rep{_rep}", [T_LOC, H], F32).ap()
                env["do_warmup"] = _rep == 0
                _emit_body(nc, tc, env)
    return nc


def _emit_body(nc, tc, pools):
    const = pools["const"]; xload = pools["xload"]; xqp = pools["xq"] if "xq" in pools else pools["xqp"]
    xqp = pools["xqp"]; xqtp = pools["xqtp"]; small = pools["small"]
    wload = pools["wload"]; wcast = pools["wcast"]; scalep = pools["scalep"]
    hbuf = pools["hbuf"]; hqp = pools["hqp"]
    outp = pools["outp"]; ptp = pools["ptp"]; pgp = pools["pgp"]
    pup = pools["pup"]; p2p = pools["p2p"]
    x_d = pools["x_d"]; w13T_d = pools["w13T_d"]; w2T_d = pools["w2T_d"]
    xT_d = pools["xT_d"]
    wsg_d = pools["wsg_d"]; wsu_d = pools["wsu_d"]; w2s_d = pools["w2s_d"]
    out_d = pools["out_d"]
    ident = pools["ident"]
    ident_f32 = pools["ident_f32"]
    ones_row = pools["ones_row"]

    xqT = {}     # t-block -> [128h, KT1, 128t] bf16
    s1s = {}     # t-block -> [128, 1] f32 quant scale
    cast_n = [0]

    def warmup():
        # dependency-free matmuls to pull the PE HAM clock to 2.4 GHz
        # while the lead-in DMAs run (rep 0 only)
        pw = ptp.tile([128, 128], F32, tag="pt", name="warm")
        for _ in range(100):
            nc.tensor.matmul(pw[:], ident[:], ident[:], start=True,
                             stop=True)

    def cast(dst, src):
        _cast_engine(nc, cast_n[0])(dst, src)
        cast_n[0] += 1

    invbs = {}

    def quantize_scales(tb):
        # amax over the natural-layout row chunks (free-dim reduce)
        NQ = 8
        QW = H // NQ
        am = None
        for hh in range(NQ):
            xt = xload.tile([128, QW], F32, tag="xt", name=f"xt{tb}_{hh}",
                            bufs=2)
            nc.sync.dma_start(
                xt[:], x_d[tb * 128:(tb + 1) * 128, hh * QW:(hh + 1) * QW])
            amn = small.tile([128, 1], F32, tag="amax1", name=f"am{tb}_{hh}")
            nc.vector.tensor_reduce(amn[:], xt[:], axis=AX.X, op=ALU.max,
                                    apply_absolute_value=True)
            if hh > 0:
                am2 = small.tile([128, 1], F32, tag="amax1b",
                                 name=f"amc{tb}_{hh}")
                nc.vector.tensor_tensor(am2[:], am[:], amn[:], op=ALU.max)
                am = am2
            else:
                am = amn
        s1 = small.tile([128, 1], F32, tag="s1")
        nc.vector.tensor_scalar(s1[:], am[:], 1.0 / 127.0, None, op0=ALU.mult)
        inv1 = small.tile([128, 1], F32, tag="inv1")
        nc.vector.reciprocal(inv1[:], s1[:])
        s1s[tb] = s1
        # pre-issue the first xT tile so the quant STT can fire the moment
        # the scale is ready
        KQ = KT1 // 4
        QWA = H // 4
        xTt0 = xload.tile([128, KQ, 128], F32, tag="xTt",
                          name=f"xTt{tb}_0", bufs=3)
        nc.sync.dma_start(
            xTt0[:],
            xT_d[0:QWA, tb * 128:(tb + 1) * 128]
            .rearrange("(k p) t -> p k t", p=128))
        return inv1, xTt0

    def quantize_bounce(tb, inv1):
        # broadcast inv1 across partitions with a PE outer product
        pinv = ptp.tile([128, 128], F32, tag="pt", name="pinv")
        nc.tensor.transpose(pinv[0:1, :], inv1[:], ident_f32[:])
        invrow = small.tile([128, 128], F32, tag="invrow")
        nc.vector.tensor_copy(invrow[0:1, :], pinv[0:1, :])
        pbc = ptp.tile([128, 128], F32, tag="pt", name="pbc")
        nc.tensor.matmul(pbc[:], ones_row[0:1, :], invrow[0:1, :],
                         start=True, stop=True)
        invb = small.tile([128, 1, 128], F32, tag="invb")
        nc.vector.tensor_copy(invb[:, 0, :], pbc[:])
        invbs[tb] = invb

    def quantize_apply(tb, xTt0):
        # quantize the host-pretransposed xT directly in [h, t] layout:
        # multiply by the broadcast scale, round to int8, cast to bf16.
        NQ = 4
        QW = H // NQ
        KQ = KT1 // NQ
        invb = invbs[tb]
        xqt = xqtp.tile([128, KT1, 128], BF16, tag="xqT")
        for hh in range(NQ):
            if hh == 0:
                xTt = xTt0
            else:
                xTt = xload.tile([128, KQ, 128], F32, tag="xTt",
                                 name=f"xTt{tb}_{hh}", bufs=3)
                nc.sync.dma_start(
                    xTt[:],
                    xT_d[hh * QW:(hh + 1) * QW, tb * 128:(tb + 1) * 128]
                    .rearrange("(k p) t -> p k t", p=128))
            xq8 = xqp.tile([128, KQ, 128], I8, tag="xq_i8",
                           name=f"xq8_{hh}", bufs=3)
            nc.vector.scalar_tensor_tensor(
                xq8[:], xTt[:], 1.0, invb[:].broadcast_to([128, KQ, 128]),
                op0=ALU.mult, op1=ALU.mult)
            nc.scalar.copy(xqt[:, hh * KQ:(hh + 1) * KQ, :], xq8[:])
        xqT[tb] = xqt

    def mm1_loads(e, c0, cw):
        wg_i8 = [wload.tile([128, KT1 // 2, cw], I8, tag="wg_i8",
                            name=f"wg_i8_{e}_{c0}_{h2}") for h2 in range(2)]
        wu_i8 = [wload.tile([128, KT1 // 2, cw], I8, tag="wu_i8",
                            name=f"wu_i8_{e}_{c0}_{h2}") for h2 in range(2)]
        g_src = w13T_d[e, :, c0:c0 + cw].rearrange("(k p) o -> p k o", p=128)
        u_src = w13T_d[e, :, I + c0:I + c0 + cw].rearrange(
            "(k p) o -> p k o", p=128)
        for h2 in range(2):
            ksl = slice(h2 * (KT1 // 2), (h2 + 1) * (KT1 // 2))
            nc.sync.dma_start(wg_i8[h2][:], g_src[:, ksl, :])
            nc.sync.dma_start(wu_i8[h2][:], u_src[:, ksl, :])
        return wg_i8, wu_i8

    QK = 4  # k-tiles per cast op

    def cast_quad(w_i8, kq, cw, nm):
        h2, kkq = divmod(kq, (KT1 // 2) // QK)
        ks = slice(kkq * QK, (kkq + 1) * QK)
        w_bf = wcast.tile([128, QK, cw], BF16, tag="wbf", name=nm)
        cast(w_bf[:], w_i8[h2][:, ks, :])
        return w_bf

    def mm1_epilogue(e, i_tb, tb, c0, cw, pg, pu, wsg, wsu, hts, amaxes):
        gate = outp.tile([128, cw], F32, tag="gate")
        nc.vector.scalar_tensor_tensor(
            gate[:], pg, s1s[tb][:], wsg[:, c0:c0 + cw],
            op0=ALU.mult, op1=ALU.mult)
        up = outp.tile([128, cw], F32, tag="up")
        nc.vector.scalar_tensor_tensor(
            up[:], pu, s1s[tb][:], wsu[:, c0:c0 + cw],
            op0=ALU.mult, op1=ALU.mult)
        sg = outp.tile([128, cw], F32, tag="sg")
        nc.scalar.activation(sg[:], gate[:], AF.Silu)
        nc.vector.tensor_mul(hts[i_tb][:, c0:c0 + cw], sg[:], up[:])
        # per-chunk partial abs-max keeps the requant scale off the
        # critical path (ready right after the last chunk's h lands)
        prev = amaxes[i_tb]
        amp = small.tile([128, 1], F32, tag="amax2", name=f"am2p_{i_tb}_{c0}")
        nc.vector.tensor_reduce(amp[:], hts[i_tb][:, c0:c0 + cw],
                                axis=AX.X, op=ALU.max,
                                apply_absolute_value=True)
        if prev is not None:
            amn = small.tile([128, 1], F32, tag="amax2",
                             name=f"am2_{i_tb}_{c0}")
            nc.vector.tensor_tensor(amn[:], prev[:], amp[:], op=ALU.max)
            amaxes[i_tb] = amn
        else:
            amaxes[i_tb] = amp

    def mm1_chunk(e, tbs, c0, cw, wsg, wsu, hts, amaxes, loads,
                  precast=None):
        wg_i8, wu_i8 = loads
        pg = [pgp.tile([128, cw], F32, tag="pg", name=f"pg{i}")
              for i in range(len(tbs))]
        pu = [pup.tile([128, cw], F32, tag="pu", name=f"pu{i}")
              for i in range(len(tbs))]
        for kq in range(KT1 // QK):
            if precast is not None and kq < len(precast):
                wg_bf, wu_bf = precast[kq]
            else:
                wg_bf = cast_quad(wg_i8, kq, cw, "wg_bf")
                wu_bf = cast_quad(wu_i8, kq, cw, "wu_bf")
            for dk in range(QK):
                k = kq * QK + dk
                st, sp = (k == 0), (k == KT1 - 1)
                for i_tb, tb in enumerate(tbs):
                    nc.tensor.matmul(pg[i_tb][:], xqT[tb][:, k, :],
                                     wg_bf[:, dk, :], start=st, stop=sp)
                    nc.tensor.matmul(pu[i_tb][:], xqT[tb][:, k, :],
                                     wu_bf[:, dk, :], start=st, stop=sp)
        for i_tb, tb in enumerate(tbs):
            mm1_epilogue(e, i_tb, tb, c0, cw, pg[i_tb][:], pu[i_tb][:],
                         wsg, wsu, hts, amaxes)

    def mm1_chunk_split(e, tbs, c0, cw, wsg, wsu, hts, amaxes, loads, hqT,
                        s2s):
        # last chunk: separate per-tb passes (own casts) so tb0's requant
        # chain runs under tb1's matmuls instead of stalling the PE
        wg_i8, wu_i8 = loads
        for i_tb, tb in enumerate(tbs):
            pg = pgp.tile([128, cw], F32, tag="pg", name=f"pgs{i_tb}")
            pu = pup.tile([128, cw], F32, tag="pu", name=f"pus{i_tb}")
            for kq in range(KT1 // QK):
                wg_bf = cast_quad(wg_i8, kq, cw, "wg_bf")
                wu_bf = cast_quad(wu_i8, kq, cw, "wu_bf")
                for dk in range(QK):
                    k = kq * QK + dk
                    st, sp = (k == 0), (k == KT1 - 1)
                    nc.tensor.matmul(pg[:], xqT[tb][:, k, :],
                                     wg_bf[:, dk, :], start=st, stop=sp)
                    nc.tensor.matmul(pu[:], xqT[tb][:, k, :],
                                     wu_bf[:, dk, :], start=st, stop=sp)
            mm1_epilogue(e, i_tb, tb, c0, cw, pg[:], pu[:], wsg, wsu, hts,
                         amaxes)
            hqt, s2 = requant_tb(hts[i_tb], amaxes[i_tb])
            hqT.append(hqt)
            s2s.append(s2)

    def requant_dve(ht, amax2):
        s2 = small.tile([128, 1], F32, tag="s2")
        nc.vector.tensor_scalar(s2[:], amax2[:], 1.0 / 127.0, None,
                                op0=ALU.mult)
        inv2 = small.tile([128, 1], F32, tag="inv2")
        nc.vector.reciprocal(inv2[:], s2[:])
        hq_i8 = hqp.tile([128, I], I8, tag="hq_i8")
        hq_bf = hqp.tile([128, I], BF16, tag="hq_bf")
        for a, b in ((0, 512), (512, I)):
            nc.vector.tensor_scalar(hq_i8[:, a:b], ht[:, a:b], inv2[:], None,
                                    op0=ALU.mult)
            nc.scalar.copy(hq_bf[:, a:b], hq_i8[:, a:b])
        return hq_bf, s2

    def requant_pe(hq_bf):
        # [t, i] -> [i, t] via the DMA XBAR (ACT HWDGE ring): slower than
        # PE per byte (~110 GB/s) but takes 11*128 cycles off the PE and
        # has plenty of latency slack before mm2 needs it
        hqt = hqp.tile([128, KT2, 128], BF16, tag="hqT", bufs=3)
        nc.scalar.dma_start_transpose(hqt[:, :, :], hq_bf[:, :])
        return hqt

    def mm2_chunk(e, tbs, c0, cw, hqT, s2s, w2s_unused=None):
        w2sc = outp.tile([128, cw], F32, tag="w2sc", bufs=4)
        nc.sync.dma_start(w2sc[:], w2s_d[e, :, c0:c0 + cw])
        w2_i8 = wload.tile([128, KT2, cw], I8, tag="w2_i8")
        nc.sync.dma_start(
            w2_i8[:],
            w2T_d[e, :, c0:c0 + cw].rearrange("(k p) o -> p k o", p=128))
        p2 = [p2p.tile([128, cw], F32, tag="p2", name=f"p2_{i}")
              for i in range(len(tbs))]
        k = 0
        for qn in (4, 4, 3):
            w2_bf = wcast.tile([128, qn, cw], BF16, tag="wbf", name="w2_bf")
            cast(w2_bf[:], w2_i8[:, k:k + qn, :])
            for dk in range(qn):
                for i_tb in range(2):
                    nc.tensor.matmul(p2[i_tb][:], hqT[i_tb][:, k, :],
                                     w2_bf[:, dk, :], start=(k == 0),
                                     stop=(k == KT2 - 1))
                k += 1
        for i_tb, tb in enumerate(tbs):
            ot = outp.tile([128, cw], F32, tag="ot", bufs=4)
            nc.vector.scalar_tensor_tensor(
                ot[:], p2[i_tb][:], s2s[i_tb][:], w2sc[:],
                op0=ALU.mult, op1=ALU.mult)
            nc.scalar.dma_start(out_d[tb * 128:(tb + 1) * 128, c0:c0 + cw],
                                ot[:])

    # ---- Staged two-expert pipeline ----
    # Emission order == engine-queue order, so stages are interleaved to keep
    # the PE fed across quant/requant latency chains.
    assert E_LOC == 2
    loads_ = {}
    wsgs, wsus, htss, amaxs = {}, {}, {}, {}
    hqbfs, s2ss, hqTs = {}, {}, {}

    def mm1_front(e):
        wsg = scalep.tile([128, I], F32, tag="wsg", name=f"wsg{e}")
        nc.sync.dma_start(wsg[:], wsg_d[e])
        wsu = scalep.tile([128, I], F32, tag="wsu", name=f"wsu{e}")
        nc.sync.dma_start(wsu[:], wsu_d[e])
        wsgs[e], wsus[e] = wsg, wsu
        htss[e] = [hbuf.tile([128, I], F32, tag="ht", name=f"ht{e}_{i}")
                   for i in range(2)]
        amaxs[e] = [None, None]

    def mm1_run_chunk(e, ci, precast=None):
        tbs = [2 * e, 2 * e + 1]
        c0, cw = I_CHUNKS[ci]
        ld = loads_.pop((e, ci), None)
        if ld is None:
            ld = mm1_loads(e, c0, cw)
        mm1_chunk(e, tbs, c0, cw, wsgs[e], wsus[e], htss[e], amaxs[e], ld,
                  precast=precast)

    def req_dve(e):
        hqbfs[e], s2ss[e] = [], []
        for i_tb in range(2):
            hq_bf, s2 = requant_dve(htss[e][i_tb], amaxs[e][i_tb])
            hqbfs[e].append(hq_bf)
            s2ss[e].append(s2)

    def req_pe(e):
        hqTs[e] = [requant_pe(hqbfs[e][i_tb]) for i_tb in range(2)]

    # --- expert 0 front: tb0's amax stream first, then weights ---
    if pools.get("do_warmup", True):
        warmup()
    inv_a, xt0_a = quantize_scales(0)
    loads_[(0, 0)] = mm1_loads(0, *I_CHUNKS[0])
    precast0 = [(cast_quad(loads_[(0, 0)][0], kq, I_CHUNKS[0][1], "wg_bf"),
                 cast_quad(loads_[(0, 0)][1], kq, I_CHUNKS[0][1], "wu_bf"))
                for kq in range(3)]
    inv_b, xt0_b = quantize_scales(1)
    quantize_bounce(0, inv_a)
    quantize_apply(0, xt0_a)
    quantize_bounce(1, inv_b)
    quantize_apply(1, xt0_b)
    mm1_front(0)
    mm1_run_chunk(0, 0, precast=precast0)
    # expert-1 token quant hides under expert-0 mm1
    inv_c, xt0_c = quantize_scales(2)
    inv_d, xt0_d = quantize_scales(3)
    mm1_run_chunk(0, 1)
    quantize_bounce(2, inv_c)
    quantize_bounce(3, inv_d)
    quantize_apply(2, xt0_c)
    quantize_apply(3, xt0_d)
    mm1_front(1)
    mm1_run_chunk(0, 2)
    # requant scale chain (DVE/ACT only) for e0, then e1 matmuls keep PE busy
    req_dve(0)
    mm1_run_chunk(1, 0)
    # e0 hq transposes: hq_bf has long been ready by now
    req_pe(0)
    for ci in range(5):
        mm2_chunk(0, [0, 1], *H_CHUNKS[ci], hqTs[0], s2ss[0])
    mm1_run_chunk(1, 1)
    mm1_run_chunk(1, 2)
    req_dve(1)
    # e0's mm2 tail covers e1's requant chain
    for ci in range(5, 8):
        mm2_chunk(0, [0, 1], *H_CHUNKS[ci], hqTs[0], s2ss[0])
    req_pe(1)
    for ci in range(8):
        mm2_chunk(1, [2, 3], *H_CHUNKS[ci], hqTs[1], s2ss[1])


_cached_nc = None


def _make_in_maps(x, w13, w2, w13_scale, smooth_scale_2, w2_scale):
    x = np.asarray(x, dtype=np.float32)
    w13 = np.asarray(w13).astype(np.int8, copy=False)
    w2 = np.asarray(w2).astype(np.int8, copy=False)
    w13_scale = np.asarray(w13_scale, dtype=np.float32)
    smooth_scale_2 = np.asarray(smooth_scale_2, dtype=np.float32)
    w2_scale = np.asarray(w2_scale, dtype=np.float32)

    # Fold the (linear) smooth scale into the up-projection dequant scale.
    wsu_full = w13_scale[:, I:] * smooth_scale_2          # [E, I]
    wsg_full = w13_scale[:, :I]                           # [E, I]

    in_maps = []
    for c in range(NCORES):
        es = slice(E_LOC * c, E_LOC * (c + 1))
        ts = slice(T_LOC * c, T_LOC * (c + 1))
        in_maps.append({
            "x": np.ascontiguousarray(x[ts]),
            "xT": np.ascontiguousarray(x[ts].T),
            "w13T": np.ascontiguousarray(w13[es].transpose(0, 2, 1)),
            "w2T": np.ascontiguousarray(w2[es].transpose(0, 2, 1)),
            "wsg": np.ascontiguousarray(
                np.broadcast_to(wsg_full[es, None, :], (E_LOC, 128, I))),
            "wsu": np.ascontiguousarray(
                np.broadcast_to(wsu_full[es, None, :], (E_LOC, 128, I))),
            "w2s": np.ascontiguousarray(
                np.broadcast_to(w2_scale[es, None, :], (E_LOC, 128, H))),
        })
    return in_maps


def _run(in_maps, **kwargs):
    global _cached_nc
    _install_compile_hook()
    if _cached_nc is None:
        _cached_nc = _build_program()
    return run_bass_kernel_spmd(_cached_nc, in_maps, list(range(NCORES)),
                                **kwargs)


def kernel(x, w13, w2, w13_scale, smooth_scale_2, w2_scale, expert_tokens):
    # expert_tokens describes the fixed equal contiguous grouping (the
    # reference ignores it); we rely on that same grouping.
    del expert_tokens
    in_maps = _make_in_maps(x, w13, w2, w13_scale, smooth_scale_2, w2_scale)
    res = _run(in_maps)
    return np.concatenate([res.results[c]["out"] for c in range(NCORES)],
                          axis=0)


def run_profiled(x, w13, w2, w13_scale, smooth_scale_2, w2_scale,
                 expert_tokens):
    """test.py helper: run with NTFF profiling, return BassKernelResults."""
    del expert_tokens
    in_maps = _make_in_maps(x, w13, w2, w13_scale, smooth_scale_2, w2_scale)
    return _run(in_maps, trace=True)



# revision 10
# speedup vs baseline: 1.5322x; 1.3712x over previous
"""DynamicA8W8 MoE FFN on 8 TRN2 NeuronCores.

Sizes (hardcoded from the problem spec):
  T=4096 tokens, H=4096 hidden, I=1408 intermediate, E=16 experts,
  equal contiguous groups of TPE=256 tokens per expert.

Sharding: expert-parallel == token-parallel here (contiguous equal groups).
Core c owns experts {2c, 2c+1} and tokens [512c, 512c+512). No cross-core
communication is needed; each core computes its own [512, H] output slab and
the host concatenates.

Per-core pipeline:
  1. per-token dynamic quant of x -> int8 (RNE+saturate via f32->int8 copy),
     exact in bf16; PE-transpose to [h, t] layout for use as matmul stationary.
  2. grouped GEMM1 vs w13 (int8 weights DMA'd raw, cast to bf16 on chip;
     bf16 matmul is exact for int8 operands, fp32 PSUM accumulate).
  3. SwiGLU epilogue fused with dequant scales, dynamic requant to int8.
  4. GEMM2 vs w2, fused per-channel + per-token dequant, DMA out.
"""

import json

import numpy as np

import concourse.bass as bass
import concourse.bass2jax as bass2jax
import concourse.mybir as mybir
from concourse.bass_utils import run_bass_kernel_spmd
from concourse.masks import make_identity
from concourse.tile import TileContext

F32 = mybir.dt.float32
BF16 = mybir.dt.bfloat16
I8 = mybir.dt.int8
AF = mybir.ActivationFunctionType
ALU = mybir.AluOpType
AX = mybir.AxisListType

import os as _os
def _cfg(name, default):
    return int(_os.environ.get("K_" + name, default))
HQXBAR = _cfg("HQXBAR", 1)   # hq transpose via DMA XBAR (else PE)
STACT = _cfg("STACT", 1)     # out stores on ACT ring (else SP)
WARM = _cfg("WARM", 1)       # warm-up matmuls in rep 0

T, H, I, E = 4096, 4096, 1408, 16
NCORES = 8
E_LOC = E // NCORES            # 2 experts per core
TPE = T // E                   # 256 tokens per expert
T_LOC = E_LOC * TPE            # 512 tokens per core
NTB = T_LOC // 128             # 4 token blocks per core
KT1 = H // 128                 # 32 k-tiles for mm1
KT2 = I // 128                 # 11 k-tiles for mm2
# gate/up column chunks (free dim of mm1, <=512 per PSUM bank)
I_CHUNKS = [(0, 512), (512, 512), (1024, 384)]
H_CHUNKS = [(c, 512) for c in range(0, H, 512)]


# --- walrus workaround: this build rejects >1 sync wait per instruction.
# Split extras into standalone single-wait EventSemaphore instructions placed
# immediately before, on the same engine queue.
def _split_multi_waits(bir_json: bytes) -> bytes:
    j = json.loads(bir_json)
    changed = False
    for fn in j.get("functions", []):
        for blk in fn.get("blocks", []):
            out = []
            for inst in blk.get("instructions", []):
                si = inst.get("sync_info")
                waits = si.get("on_wait") if si else None
                if waits and len(waits) > 1:
                    spill, keep = waits[:-1], waits[-1:]
                    for k, w in enumerate(spill):
                        out.append({
                            "debug": inst.get("debug", 0),
                            "engine": inst["engine"],
                            "ins": [], "outs": [],
                            "name": f"{inst['name']}_w{k}",
                            "opcode": "EventSemaphore",
                            "sync_info": {"on_update": [], "on_wait": [w]},
                        })
                    si["on_wait"] = keep
                    changed = True
                out.append(inst)
            blk["instructions"] = out
    return json.dumps(j).encode() if changed else bir_json


_hook_installed = False


def _install_compile_hook():
    global _hook_installed
    if _hook_installed:
        return
    orig = bass2jax.compile_bir_kernel

    def wrapped(bir_json, tmpdir, neff_name="file.neff"):
        return orig(_split_multi_waits(bir_json), tmpdir, neff_name=neff_name)

    bass2jax.compile_bir_kernel = wrapped
    _hook_installed = True


def _cast_engine(nc, idx):
    """Round-robin the int8->bf16 weight casts across ACT/Pool/DVE.

    Balance for the engine rates (ACT 1.2G, DVE 0.96G, Pool ~0.72G effective)
    and each engine's other work: ACT 3/8, Pool 3/8, DVE 2/8.
    """
    # HW-measured int8->bf16 rates (ns per lane-elem): ACT 0.86, DVE 0.78,
    # Pool 3.9 (gpsimd is ~4x slower than the cost model thinks, and one slow
    # cast on the critical path stalls 8 matmuls) -- so no Pool casts at all.
    r = idx % 9
    if r < 5:
        return nc.scalar.copy
    return nc.vector.tensor_copy


def _build_program(reps=1):
    nc = bass.Bass()
    x_d = nc.declare_dram_parameter("x", [T_LOC, H], F32, isOutput=False)
    xT_d = nc.declare_dram_parameter("xT", [H, T_LOC], F32, isOutput=False)
    w13T_d = nc.declare_dram_parameter("w13T", [E_LOC, H, 2 * I], I8, isOutput=False)
    w2T_d = nc.declare_dram_parameter("w2T", [E_LOC, I, H], I8, isOutput=False)
    wsg_d = nc.declare_dram_parameter("wsg", [E_LOC, 128, I], F32, isOutput=False)
    wsu_d = nc.declare_dram_parameter("wsu", [E_LOC, 128, I], F32, isOutput=False)
    w2s_d = nc.declare_dram_parameter("w2s", [E_LOC, 128, H], F32, isOutput=False)
    out_d = nc.declare_dram_parameter("out", [T_LOC, H], F32, isOutput=True)

    with TileContext(nc) as tc:
        with (
            tc.tile_pool(name="const", bufs=1) as const,
            tc.tile_pool(name="xload", bufs=4) as xload,
            tc.tile_pool(name="xq", bufs=1) as xqp,
            tc.tile_pool(name="xqt", bufs=4) as xqtp,
            tc.tile_pool(name="small", bufs=4) as small,
            tc.tile_pool(name="wload", bufs=2) as wload,
            tc.tile_pool(name="wcast", bufs=7) as wcast,
            tc.tile_pool(name="scales", bufs=2) as scalep,
            tc.tile_pool(name="hbuf", bufs=2) as hbuf,
            tc.tile_pool(name="hq", bufs=2) as hqp,
            tc.tile_pool(name="outp", bufs=2) as outp,
            tc.tile_pool(name="pt", bufs=2, space="PSUM") as ptp,
            tc.tile_pool(name="pg", bufs=2, space="PSUM") as pgp,
            tc.tile_pool(name="pu", bufs=2, space="PSUM") as pup,
            tc.tile_pool(name="p2", bufs=2, space="PSUM") as p2p,
        ):
            env = dict(locals())
            ident = const.tile([128, 128], BF16)
            make_identity(nc, ident)
            env["ident"] = ident
            ident_f32 = const.tile([128, 128], F32)
            make_identity(nc, ident_f32)
            env["ident_f32"] = ident_f32
            ones_row = const.tile([128, 128], F32)
            nc.vector.memset(ones_row[:], 1.0)
            env["ones_row"] = ones_row
            for _rep in range(reps):
                if _rep > 0:
                    env["out_d"] = nc.dram_tensor(
                        f"out_rep{_rep}", [T_LOC, H], F32).ap()
                env["do_warmup"] = _rep == 0
                _emit_body(nc, tc, env)
    return nc


def _emit_body(nc, tc, pools):
    const = pools["const"]; xload = pools["xload"]; xqp = pools["xq"] if "xq" in pools else pools["xqp"]
    xqp = pools["xqp"]; xqtp = pools["xqtp"]; small = pools["small"]
    wload = pools["wload"]; wcast = pools["wcast"]; scalep = pools["scalep"]
    hbuf = pools["hbuf"]; hqp = pools["hqp"]
    outp = pools["outp"]; ptp = pools["ptp"]; pgp = pools["pgp"]
    pup = pools["pup"]; p2p = pools["p2p"]
    x_d = pools["x_d"]; w13T_d = pools["w13T_d"]; w2T_d = pools["w2T_d"]
    xT_d = pools["xT_d"]
    wsg_d = pools["wsg_d"]; wsu_d = pools["wsu_d"]; w2s_d = pools["w2s_d"]
    out_d = pools["out_d"]
    ident = pools["ident"]
    ident_f32 = pools["ident_f32"]
    ones_row = pools["ones_row"]

    xqT = {}     # t-block -> [128h, KT1, 128t] bf16
    s1s = {}     # t-block -> [128, 1] f32 quant scale
    cast_n = [0]

    def warmup():
        # dependency-free matmuls to pull the PE HAM clock to 2.4 GHz
        # while the lead-in DMAs run (rep 0 only)
        pw = ptp.tile([128, 128], F32, tag="pt", name="warm")
        for _ in range(100):
            nc.tensor.matmul(pw[:], ident[:], ident[:], start=True,
                             stop=True)

    def cast(dst, src):
        _cast_engine(nc, cast_n[0])(dst, src)
        cast_n[0] += 1

    invbs = {}

    def quantize_scales(tb):
        # amax over the natural-layout row chunks (free-dim reduce)
        NQ = 8
        QW = H // NQ
        am = None
        for hh in range(NQ):
            xt = xload.tile([128, QW], F32, tag="xt", name=f"xt{tb}_{hh}",
                            bufs=2)
            nc.sync.dma_start(
                xt[:], x_d[tb * 128:(tb + 1) * 128, hh * QW:(hh + 1) * QW])
            amn = small.tile([128, 1], F32, tag="amax1", name=f"am{tb}_{hh}")
            nc.vector.tensor_reduce(amn[:], xt[:], axis=AX.X, op=ALU.max,
                                    apply_absolute_value=True)
            if hh > 0:
                am2 = small.tile([128, 1], F32, tag="amax1b",
                                 name=f"amc{tb}_{hh}")
                nc.vector.tensor_tensor(am2[:], am[:], amn[:], op=ALU.max)
                am = am2
            else:
                am = amn
        s1 = small.tile([128, 1], F32, tag="s1")
        nc.vector.tensor_scalar(s1[:], am[:], 1.0 / 127.0, None, op0=ALU.mult)
        inv1 = small.tile([128, 1], F32, tag="inv1")
        nc.vector.reciprocal(inv1[:], s1[:])
        s1s[tb] = s1
        # pre-issue the first xT tile so the quant STT can fire the moment
        # the scale is ready
        KQ = KT1 // 4
        QWA = H // 4
        xTt0 = xload.tile([128, KQ, 128], F32, tag="xTt",
                          name=f"xTt{tb}_0", bufs=3)
        nc.sync.dma_start(
            xTt0[:],
            xT_d[0:QWA, tb * 128:(tb + 1) * 128]
            .rearrange("(k p) t -> p k t", p=128))
        return inv1, xTt0

    def quantize_bounce(tb, inv1):
        # broadcast inv1 across partitions with a PE outer product
        pinv = ptp.tile([128, 128], F32, tag="pt", name="pinv")
        nc.tensor.transpose(pinv[0:1, :], inv1[:], ident_f32[:])
        invrow = small.tile([128, 128], F32, tag="invrow")
        nc.vector.tensor_copy(invrow[0:1, :], pinv[0:1, :])
        pbc = ptp.tile([128, 128], F32, tag="pt", name="pbc")
        nc.tensor.matmul(pbc[:], ones_row[0:1, :], invrow[0:1, :],
                         start=True, stop=True)
        invb = small.tile([128, 1, 128], F32, tag="invb")
        nc.vector.tensor_copy(invb[:, 0, :], pbc[:])
        invbs[tb] = invb

    def quantize_apply(tb, xTt0):
        # quantize the host-pretransposed xT directly in [h, t] layout:
        # multiply by the broadcast scale, round to int8, cast to bf16.
        NQ = 4
        QW = H // NQ
        KQ = KT1 // NQ
        invb = invbs[tb]
        xqt = xqtp.tile([128, KT1, 128], BF16, tag="xqT")
        for hh in range(NQ):
            if hh == 0:
                xTt = xTt0
            else:
                xTt = xload.tile([128, KQ, 128], F32, tag="xTt",
                                 name=f"xTt{tb}_{hh}", bufs=3)
                nc.sync.dma_start(
                    xTt[:],
                    xT_d[hh * QW:(hh + 1) * QW, tb * 128:(tb + 1) * 128]
                    .rearrange("(k p) t -> p k t", p=128))
            xq8 = xqp.tile([128, KQ, 128], I8, tag="xq_i8",
                           name=f"xq8_{hh}", bufs=3)
            nc.vector.scalar_tensor_tensor(
                xq8[:], xTt[:], 1.0, invb[:].broadcast_to([128, KQ, 128]),
                op0=ALU.mult, op1=ALU.mult)
            nc.scalar.copy(xqt[:, hh * KQ:(hh + 1) * KQ, :], xq8[:])
        xqT[tb] = xqt

    def mm1_loads(e, c0, cw):
        wg_i8 = [wload.tile([128, KT1 // 2, cw], I8, tag="wg_i8",
                            name=f"wg_i8_{e}_{c0}_{h2}") for h2 in range(2)]
        wu_i8 = [wload.tile([128, KT1 // 2, cw], I8, tag="wu_i8",
                            name=f"wu_i8_{e}_{c0}_{h2}") for h2 in range(2)]
        g_src = w13T_d[e, :, c0:c0 + cw].rearrange("(k p) o -> p k o", p=128)
        u_src = w13T_d[e, :, I + c0:I + c0 + cw].rearrange(
            "(k p) o -> p k o", p=128)
        for h2 in range(2):
            ksl = slice(h2 * (KT1 // 2), (h2 + 1) * (KT1 // 2))
            nc.sync.dma_start(wg_i8[h2][:], g_src[:, ksl, :])
            nc.sync.dma_start(wu_i8[h2][:], u_src[:, ksl, :])
        return wg_i8, wu_i8

    QK = 4  # k-tiles per cast op

    def cast_quad(w_i8, kq, cw, nm):
        h2, kkq = divmod(kq, (KT1 // 2) // QK)
        ks = slice(kkq * QK, (kkq + 1) * QK)
        w_bf = wcast.tile([128, QK, cw], BF16, tag="wbf", name=nm)
        cast(w_bf[:], w_i8[h2][:, ks, :])
        return w_bf

    def mm1_epilogue(e, i_tb, tb, c0, cw, pg, pu, wsg, wsu, hts, amaxes):
        gate = outp.tile([128, cw], F32, tag="gate")
        nc.vector.scalar_tensor_tensor(
            gate[:], pg, s1s[tb][:], wsg[:, c0:c0 + cw],
            op0=ALU.mult, op1=ALU.mult)
        up = outp.tile([128, cw], F32, tag="up")
        nc.vector.scalar_tensor_tensor(
            up[:], pu, s1s[tb][:], wsu[:, c0:c0 + cw],
            op0=ALU.mult, op1=ALU.mult)
        sg = outp.tile([128, cw], F32, tag="sg")
        nc.scalar.activation(sg[:], gate[:], AF.Silu)
        nc.vector.tensor_mul(hts[i_tb][:, c0:c0 + cw], sg[:], up[:])
        # per-chunk partial abs-max keeps the requant scale off the
        # critical path (ready right after the last chunk's h lands)
        prev = amaxes[i_tb]
        amp = small.tile([128, 1], F32, tag="amax2", name=f"am2p_{i_tb}_{c0}")
        nc.vector.tensor_reduce(amp[:], hts[i_tb][:, c0:c0 + cw],
                                axis=AX.X, op=ALU.max,
                                apply_absolute_value=True)
        if prev is not None:
            amn = small.tile([128, 1], F32, tag="amax2",
                             name=f"am2_{i_tb}_{c0}")
            nc.vector.tensor_tensor(amn[:], prev[:], amp[:], op=ALU.max)
            amaxes[i_tb] = amn
        else:
            amaxes[i_tb] = amp

    def mm1_chunk(e, tbs, c0, cw, wsg, wsu, hts, amaxes, loads,
                  precast=None):
        wg_i8, wu_i8 = loads
        pg = [pgp.tile([128, cw], F32, tag="pg", name=f"pg{i}")
              for i in range(len(tbs))]
        pu = [pup.tile([128, cw], F32, tag="pu", name=f"pu{i}")
              for i in range(len(tbs))]
        for kq in range(KT1 // QK):
            if precast is not None and kq < len(precast):
                wg_bf, wu_bf = precast[kq]
            else:
                wg_bf = cast_quad(wg_i8, kq, cw, "wg_bf")
                wu_bf = cast_quad(wu_i8, kq, cw, "wu_bf")
            for dk in range(QK):
                k = kq * QK + dk
                st, sp = (k == 0), (k == KT1 - 1)
                for i_tb, tb in enumerate(tbs):
                    nc.tensor.matmul(pg[i_tb][:], xqT[tb][:, k, :],
                                     wg_bf[:, dk, :], start=st, stop=sp)
                    nc.tensor.matmul(pu[i_tb][:], xqT[tb][:, k, :],
                                     wu_bf[:, dk, :], start=st, stop=sp)
        for i_tb, tb in enumerate(tbs):
            mm1_epilogue(e, i_tb, tb, c0, cw, pg[i_tb][:], pu[i_tb][:],
                         wsg, wsu, hts, amaxes)

    def mm1_chunk_split(e, tbs, c0, cw, wsg, wsu, hts, amaxes, loads, hqT,
                        s2s):
        # last chunk: separate per-tb passes (own casts) so tb0's requant
        # chain runs under tb1's matmuls instead of stalling the PE
        wg_i8, wu_i8 = loads
        for i_tb, tb in enumerate(tbs):
            pg = pgp.tile([128, cw], F32, tag="pg", name=f"pgs{i_tb}")
            pu = pup.tile([128, cw], F32, tag="pu", name=f"pus{i_tb}")
            for kq in range(KT1 // QK):
                wg_bf = cast_quad(wg_i8, kq, cw, "wg_bf")
                wu_bf = cast_quad(wu_i8, kq, cw, "wu_bf")
                for dk in range(QK):
                    k = kq * QK + dk
                    st, sp = (k == 0), (k == KT1 - 1)
                    nc.tensor.matmul(pg[:], xqT[tb][:, k, :],
                                     wg_bf[:, dk, :], start=st, stop=sp)
                    nc.tensor.matmul(pu[:], xqT[tb][:, k, :],
                                     wu_bf[:, dk, :], start=st, stop=sp)
            mm1_epilogue(e, i_tb, tb, c0, cw, pg[:], pu[:], wsg, wsu, hts,
                         amaxes)
            hqt, s2 = requant_tb(hts[i_tb], amaxes[i_tb])
            hqT.append(hqt)
            s2s.append(s2)

    def requant_dve(ht, amax2):
        s2 = small.tile([128, 1], F32, tag="s2")
        nc.vector.tensor_scalar(s2[:], amax2[:], 1.0 / 127.0, None,
                                op0=ALU.mult)
        inv2 = small.tile([128, 1], F32, tag="inv2")
        nc.vector.reciprocal(inv2[:], s2[:])
        hq_i8 = hqp.tile([128, I], I8, tag="hq_i8")
        hq_bf = hqp.tile([128, I], BF16, tag="hq_bf")
        for a, b in ((0, 512), (512, I)):
            nc.vector.tensor_scalar(hq_i8[:, a:b], ht[:, a:b], inv2[:], None,
                                    op0=ALU.mult)
            nc.scalar.copy(hq_bf[:, a:b], hq_i8[:, a:b])
        return hq_bf, s2

    def requant_pe(hq_bf):
        hqt = hqp.tile([128, KT2, 128], BF16, tag="hqT", bufs=3)
        if HQXBAR:
            # [t, i] -> [i, t] via the DMA XBAR (ACT HWDGE ring): slower
            # per byte (~110 GB/s) but takes 11*128 cycles off the PE
            nc.scalar.dma_start_transpose(hqt[:, :, :], hq_bf[:, :])
            return hqt
        for k0 in range(0, KT2, 4):
            kn = min(4, KT2 - k0)
            pt = ptp.tile([128, 4, 128], BF16, tag="pt", name="pt_hq")
            for dk in range(kn):
                k = k0 + dk
                nc.tensor.transpose(pt[:, dk, :],
                                    hq_bf[:, k * 128:(k + 1) * 128], ident[:])
            (nc.vector.tensor_copy if (k0 // 4) % 2 else nc.scalar.copy)(
                hqt[:, k0:k0 + kn, :], pt[:, 0:kn, :])
        return hqt

    def mm2_chunk(e, tbs, c0, cw, hqT, s2s, w2s_unused=None):
        w2sc = outp.tile([128, cw], F32, tag="w2sc", bufs=4)
        nc.sync.dma_start(w2sc[:], w2s_d[e, :, c0:c0 + cw])
        w2_i8 = wload.tile([128, KT2, cw], I8, tag="w2_i8")
        nc.sync.dma_start(
            w2_i8[:],
            w2T_d[e, :, c0:c0 + cw].rearrange("(k p) o -> p k o", p=128))
        p2 = [p2p.tile([128, cw], F32, tag="p2", name=f"p2_{i}")
              for i in range(len(tbs))]
        k = 0
        for qn in (4, 4, 3):
            w2_bf = wcast.tile([128, qn, cw], BF16, tag="wbf", name="w2_bf")
            cast(w2_bf[:], w2_i8[:, k:k + qn, :])
            for dk in range(qn):
                for i_tb in range(2):
                    nc.tensor.matmul(p2[i_tb][:], hqT[i_tb][:, k, :],
                                     w2_bf[:, dk, :], start=(k == 0),
                                     stop=(k == KT2 - 1))
                k += 1
        for i_tb, tb in enumerate(tbs):
            ot = outp.tile([128, cw], F32, tag="ot", bufs=4)
            nc.vector.scalar_tensor_tensor(
                ot[:], p2[i_tb][:], s2s[i_tb][:], w2sc[:],
                op0=ALU.mult, op1=ALU.mult)
            (nc.scalar if STACT else nc.sync).dma_start(
                out_d[tb * 128:(tb + 1) * 128, c0:c0 + cw], ot[:])

    # ---- Staged two-expert pipeline ----
    # Emission order == engine-queue order, so stages are interleaved to keep
    # the PE fed across quant/requant latency chains.
    assert E_LOC == 2
    loads_ = {}
    wsgs, wsus, htss, amaxs = {}, {}, {}, {}
    hqbfs, s2ss, hqTs = {}, {}, {}

    def mm1_front(e):
        wsg = scalep.tile([128, I], F32, tag="wsg", name=f"wsg{e}")
        nc.sync.dma_start(wsg[:], wsg_d[e])
        wsu = scalep.tile([128, I], F32, tag="wsu", name=f"wsu{e}")
        nc.sync.dma_start(wsu[:], wsu_d[e])
        wsgs[e], wsus[e] = wsg, wsu
        htss[e] = [hbuf.tile([128, I], F32, tag="ht", name=f"ht{e}_{i}")
                   for i in range(2)]
        amaxs[e] = [None, None]

    def mm1_run_chunk(e, ci, precast=None):
        tbs = [2 * e, 2 * e + 1]
        c0, cw = I_CHUNKS[ci]
        ld = loads_.pop((e, ci), None)
        if ld is None:
            ld = mm1_loads(e, c0, cw)
        mm1_chunk(e, tbs, c0, cw, wsgs[e], wsus[e], htss[e], amaxs[e], ld,
                  precast=precast)

    def req_dve(e):
        hqbfs[e], s2ss[e] = [], []
        for i_tb in range(2):
            hq_bf, s2 = requant_dve(htss[e][i_tb], amaxs[e][i_tb])
            hqbfs[e].append(hq_bf)
            s2ss[e].append(s2)

    def req_pe(e):
        hqTs[e] = [requant_pe(hqbfs[e][i_tb]) for i_tb in range(2)]

    # --- expert 0 front: tb0's amax stream first, then weights ---
    if WARM and pools.get("do_warmup", True):
        warmup()
    inv_a, xt0_a = quantize_scales(0)
    loads_[(0, 0)] = mm1_loads(0, *I_CHUNKS[0])
    precast0 = [(cast_quad(loads_[(0, 0)][0], kq, I_CHUNKS[0][1], "wg_bf"),
                 cast_quad(loads_[(0, 0)][1], kq, I_CHUNKS[0][1], "wu_bf"))
                for kq in range(3)]
    inv_b, xt0_b = quantize_scales(1)
    quantize_bounce(0, inv_a)
    quantize_apply(0, xt0_a)
    quantize_bounce(1, inv_b)
    quantize_apply(1, xt0_b)
    mm1_front(0)
    mm1_run_chunk(0, 0, precast=precast0)
    # expert-1 token quant hides under expert-0 mm1
    inv_c, xt0_c = quantize_scales(2)
    inv_d, xt0_d = quantize_scales(3)
    mm1_run_chunk(0, 1)
    quantize_bounce(2, inv_c)
    quantize_bounce(3, inv_d)
    quantize_apply(2, xt0_c)
    quantize_apply(3, xt0_d)
    mm1_front(1)
    mm1_run_chunk(0, 2)
    # requant scale chain (DVE/ACT only) for e0, then e1 matmuls keep PE busy
    req_dve(0)
    mm1_run_chunk(1, 0)
    # e0 hq transposes: hq_bf has long been ready by now
    req_pe(0)
    for ci in range(5):
        mm2_chunk(0, [0, 1], *H_CHUNKS[ci], hqTs[0], s2ss[0])
    mm1_run_chunk(1, 1)
    mm1_run_chunk(1, 2)
    req_dve(1)
    # e0's mm2 tail covers e1's requant chain
    for ci in range(5, 8):
        mm2_chunk(0, [0, 1], *H_CHUNKS[ci], hqTs[0], s2ss[0])
    req_pe(1)
    for ci in range(8):
        mm2_chunk(1, [2, 3], *H_CHUNKS[ci], hqTs[1], s2ss[1])


_cached_nc = None


def _make_in_maps(x, w13, w2, w13_scale, smooth_scale_2, w2_scale):
    x = np.asarray(x, dtype=np.float32)
    w13 = np.asarray(w13).astype(np.int8, copy=False)
    w2 = np.asarray(w2).astype(np.int8, copy=False)
    w13_scale = np.asarray(w13_scale, dtype=np.float32)
    smooth_scale_2 = np.asarray(smooth_scale_2, dtype=np.float32)
    w2_scale = np.asarray(w2_scale, dtype=np.float32)

    # Fold the (linear) smooth scale into the up-projection dequant scale.
    wsu_full = w13_scale[:, I:] * smooth_scale_2          # [E, I]
    wsg_full = w13_scale[:, :I]                           # [E, I]

    in_maps = []
    for c in range(NCORES):
        es = slice(E_LOC * c, E_LOC * (c + 1))
        ts = slice(T_LOC * c, T_LOC * (c + 1))
        in_maps.append({
            "x": np.ascontiguousarray(x[ts]),
            "xT": np.ascontiguousarray(x[ts].T),
            "w13T": np.ascontiguousarray(w13[es].transpose(0, 2, 1)),
            "w2T": np.ascontiguousarray(w2[es].transpose(0, 2, 1)),
            "wsg": np.ascontiguousarray(
                np.broadcast_to(wsg_full[es, None, :], (E_LOC, 128, I))),
            "wsu": np.ascontiguousarray(
                np.broadcast_to(wsu_full[es, None, :], (E_LOC, 128, I))),
            "w2s": np.ascontiguousarray(
                np.broadcast_to(w2_scale[es, None, :], (E_LOC, 128, H))),
        })
    return in_maps


def _run(in_maps, **kwargs):
    global _cached_nc
    _install_compile_hook()
    if _cached_nc is None:
        _cached_nc = _build_program()
    return run_bass_kernel_spmd(_cached_nc, in_maps, list(range(NCORES)),
                                **kwargs)


def kernel(x, w13, w2, w13_scale, smooth_scale_2, w2_scale, expert_tokens):
    # expert_tokens describes the fixed equal contiguous grouping (the
    # reference ignores it); we rely on that same grouping.
    del expert_tokens
    in_maps = _make_in_maps(x, w13, w2, w13_scale, smooth_scale_2, w2_scale)
    res = _run(in_maps)
    return np.concatenate([res.results[c]["out"] for c in range(NCORES)],
                          axis=0)


def run_profiled(x, w13, w2, w13_scale, smooth_scale_2, w2_scale,
                 expert_tokens):
    """test.py helper: run with NTFF profiling, return BassKernelResults."""
    del expert_tokens
    in_maps = _make_in_maps(x, w13, w2, w13_scale, smooth_scale_2, w2_scale)
    return _run(in_maps, trace=True)



# revision 14
# speedup vs baseline: 1.6591x; 1.0828x over previous
"""DynamicA8W8 MoE FFN on 8 TRN2 NeuronCores.

Sizes (hardcoded from the problem spec):
  T=4096 tokens, H=4096 hidden, I=1408 intermediate, E=16 experts,
  equal contiguous groups of TPE=256 tokens per expert.

Sharding: expert-parallel == token-parallel here (contiguous equal groups).
Core c owns experts {2c, 2c+1} and tokens [512c, 512c+512). No cross-core
communication is needed; each core computes its own [512, H] output slab and
the host concatenates.

Per-core pipeline:
  1. per-token dynamic quant of x -> int8 (RNE+saturate via f32->int8 copy),
     exact in bf16; PE-transpose to [h, t] layout for use as matmul stationary.
  2. grouped GEMM1 vs w13 (int8 weights DMA'd raw, cast to bf16 on chip;
     bf16 matmul is exact for int8 operands, fp32 PSUM accumulate).
  3. SwiGLU epilogue fused with dequant scales, dynamic requant to int8.
  4. GEMM2 vs w2, fused per-channel + per-token dequant, DMA out.
"""

import json

import numpy as np

import concourse.bass as bass
import concourse.bass2jax as bass2jax
import concourse.mybir as mybir
from concourse.bass_utils import run_bass_kernel_spmd
from concourse.masks import make_identity
from concourse.tile import TileContext

F32 = mybir.dt.float32
BF16 = mybir.dt.bfloat16
I8 = mybir.dt.int8
AF = mybir.ActivationFunctionType
ALU = mybir.AluOpType
AX = mybir.AxisListType

import os as _os
def _cfg(name, default):
    return int(_os.environ.get("K_" + name, default))
HQXBAR = _cfg("HQXBAR", 1)   # hq transpose via DMA XBAR (else PE)
PTP = _cfg("PTP", 2)         # bounce-PSUM pool bufs
P2P = _cfg("P2P", 2)         # mm2 accumulator PSUM bufs
WGB = _cfg("WGB", 2)         # wg/wu i8 load bufs per tag
CASTR = _cfg("CASTR", 54)    # cast ratio: ACT if idx%(a+d)<a for "ad"
QKC = _cfg("QKC", 4)         # k-tiles per cast op
STACT = _cfg("STACT", 0)     # out stores on ACT ring (else SP)
WARM = _cfg("WARM", 1)       # warm-up matmuls in rep 0

T, H, I, E = 4096, 4096, 1408, 16
NCORES = 8
E_LOC = E // NCORES            # 2 experts per core
TPE = T // E                   # 256 tokens per expert
T_LOC = E_LOC * TPE            # 512 tokens per core
NTB = T_LOC // 128             # 4 token blocks per core
KT1 = H // 128                 # 32 k-tiles for mm1
KT2 = I // 128                 # 11 k-tiles for mm2
# gate/up column chunks (free dim of mm1, <=512 per PSUM bank)
I_CHUNKS = [(0, 512), (512, 512), (1024, 384)]
H_CHUNKS = [(c, 512) for c in range(0, H, 512)]


# --- walrus workaround: this build rejects >1 sync wait per instruction.
# Split extras into standalone single-wait EventSemaphore instructions placed
# immediately before, on the same engine queue.
def _split_multi_waits(bir_json: bytes) -> bytes:
    j = json.loads(bir_json)
    changed = False
    for fn in j.get("functions", []):
        for blk in fn.get("blocks", []):
            out = []
            for inst in blk.get("instructions", []):
                si = inst.get("sync_info")
                waits = si.get("on_wait") if si else None
                if waits and len(waits) > 1:
                    spill, keep = waits[:-1], waits[-1:]
                    for k, w in enumerate(spill):
                        out.append({
                            "debug": inst.get("debug", 0),
                            "engine": inst["engine"],
                            "ins": [], "outs": [],
                            "name": f"{inst['name']}_w{k}",
                            "opcode": "EventSemaphore",
                            "sync_info": {"on_update": [], "on_wait": [w]},
                        })
                    si["on_wait"] = keep
                    changed = True
                out.append(inst)
            blk["instructions"] = out
    return json.dumps(j).encode() if changed else bir_json


_hook_installed = False


def _install_compile_hook():
    global _hook_installed
    if _hook_installed:
        return
    orig = bass2jax.compile_bir_kernel

    def wrapped(bir_json, tmpdir, neff_name="file.neff"):
        return orig(_split_multi_waits(bir_json), tmpdir, neff_name=neff_name)

    bass2jax.compile_bir_kernel = wrapped
    _hook_installed = True


def _cast_engine(nc, idx):
    """Round-robin the int8->bf16 weight casts across ACT/Pool/DVE.

    Balance for the engine rates (ACT 1.2G, DVE 0.96G, Pool ~0.72G effective)
    and each engine's other work: ACT 3/8, Pool 3/8, DVE 2/8.
    """
    # HW-measured int8->bf16 rates (ns per lane-elem): ACT 0.86, DVE 0.78,
    # Pool 3.9 (gpsimd is ~4x slower than the cost model thinks, and one slow
    # cast on the critical path stalls 8 matmuls) -- so no Pool casts at all.
    a, d = divmod(CASTR, 10)
    if idx % (a + d) < a:
        return nc.scalar.copy
    return nc.vector.tensor_copy


def _build_program(reps=1):
    nc = bass.Bass()
    x_d = nc.declare_dram_parameter("x", [T_LOC, H], F32, isOutput=False)
    xT_d = nc.declare_dram_parameter("xT", [H, T_LOC], F32, isOutput=False)
    w13T_d = nc.declare_dram_parameter("w13T", [E_LOC, H, 2 * I], I8, isOutput=False)
    w2T_d = nc.declare_dram_parameter("w2T", [E_LOC, I, H], I8, isOutput=False)
    wsg_d = nc.declare_dram_parameter("wsg", [E_LOC, 128, I], F32, isOutput=False)
    wsu_d = nc.declare_dram_parameter("wsu", [E_LOC, 128, I], F32, isOutput=False)
    w2s_d = nc.declare_dram_parameter("w2s", [E_LOC, 128, H], F32, isOutput=False)
    out_d = nc.declare_dram_parameter("out", [T_LOC, H], F32, isOutput=True)

    with TileContext(nc) as tc:
        with (
            tc.tile_pool(name="const", bufs=1) as const,
            tc.tile_pool(name="xload", bufs=4) as xload,
            tc.tile_pool(name="xq", bufs=1) as xqp,
            tc.tile_pool(name="xqt", bufs=4) as xqtp,
            tc.tile_pool(name="small", bufs=4) as small,
            tc.tile_pool(name="wload", bufs=2) as wload,
            tc.tile_pool(name="wcast", bufs=_cfg("WCAST", 7)) as wcast,
            tc.tile_pool(name="scales", bufs=2) as scalep,
            tc.tile_pool(name="hbuf", bufs=2) as hbuf,
            tc.tile_pool(name="hq", bufs=2) as hqp,
            tc.tile_pool(name="outp", bufs=2) as outp,
            tc.tile_pool(name="pt", bufs=PTP, space="PSUM") as ptp,
            tc.tile_pool(name="pg", bufs=2, space="PSUM") as pgp,
            tc.tile_pool(name="pu", bufs=2, space="PSUM") as pup,
            tc.tile_pool(name="p2", bufs=P2P, space="PSUM") as p2p,
        ):
            env = dict(locals())
            ident = const.tile([128, 128], BF16)
            make_identity(nc, ident)
            env["ident"] = ident
            ident_f32 = const.tile([128, 128], F32)
            make_identity(nc, ident_f32)
            env["ident_f32"] = ident_f32
            ones_row = const.tile([128, 128], F32)
            nc.vector.memset(ones_row[:], 1.0)
            env["ones_row"] = ones_row
            for _rep in range(reps):
                if _rep > 0:
                    env["out_d"] = nc.dram_tensor(
                        f"out_rep{_rep}", [T_LOC, H], F32).ap()
                env["do_warmup"] = _rep == 0
                _emit_body(nc, tc, env)
    return nc


def _emit_body(nc, tc, pools):
    const = pools["const"]; xload = pools["xload"]; xqp = pools["xq"] if "xq" in pools else pools["xqp"]
    xqp = pools["xqp"]; xqtp = pools["xqtp"]; small = pools["small"]
    wload = pools["wload"]; wcast = pools["wcast"]; scalep = pools["scalep"]
    hbuf = pools["hbuf"]; hqp = pools["hqp"]
    outp = pools["outp"]; ptp = pools["ptp"]; pgp = pools["pgp"]
    pup = pools["pup"]; p2p = pools["p2p"]
    x_d = pools["x_d"]; w13T_d = pools["w13T_d"]; w2T_d = pools["w2T_d"]
    xT_d = pools["xT_d"]
    wsg_d = pools["wsg_d"]; wsu_d = pools["wsu_d"]; w2s_d = pools["w2s_d"]
    out_d = pools["out_d"]
    ident = pools["ident"]
    ident_f32 = pools["ident_f32"]
    ones_row = pools["ones_row"]

    xqT = {}     # t-block -> [128h, KT1, 128t] bf16
    s1s = {}     # t-block -> [128, 1] f32 quant scale
    cast_n = [0]

    def warmup():
        # dependency-free matmuls to pull the PE HAM clock to 2.4 GHz
        # while the lead-in DMAs run (rep 0 only)
        pw = ptp.tile([128, 128], F32, tag="pt", name="warm")
        for _ in range(100):
            nc.tensor.matmul(pw[:], ident[:], ident[:], start=True,
                             stop=True)

    def cast(dst, src):
        _cast_engine(nc, cast_n[0])(dst, src)
        cast_n[0] += 1

    invbs = {}

    def quantize_scales(tb):
        # amax over the natural-layout row chunks (free-dim reduce)
        NQ = 8
        QW = H // NQ
        am = None
        for hh in range(NQ):
            xt = xload.tile([128, QW], F32, tag="xt", name=f"xt{tb}_{hh}",
                            bufs=2)
            nc.sync.dma_start(
                xt[:], x_d[tb * 128:(tb + 1) * 128, hh * QW:(hh + 1) * QW])
            amn = small.tile([128, 1], F32, tag="amax1", name=f"am{tb}_{hh}")
            nc.vector.tensor_reduce(amn[:], xt[:], axis=AX.X, op=ALU.max,
                                    apply_absolute_value=True)
            if hh > 0:
                am2 = small.tile([128, 1], F32, tag="amax1b",
                                 name=f"amc{tb}_{hh}")
                nc.vector.tensor_tensor(am2[:], am[:], amn[:], op=ALU.max)
                am = am2
            else:
                am = amn
        s1 = small.tile([128, 1], F32, tag="s1")
        nc.vector.tensor_scalar(s1[:], am[:], 1.0 / 127.0, None, op0=ALU.mult)
        inv1 = small.tile([128, 1], F32, tag="inv1")
        nc.vector.reciprocal(inv1[:], s1[:])
        s1s[tb] = s1
        # pre-issue the first xT tile so the quant STT can fire the moment
        # the scale is ready
        KQ = KT1 // 4
        QWA = H // 4
        xTt0 = xload.tile([128, KQ, 128], F32, tag="xTt",
                          name=f"xTt{tb}_0", bufs=3)
        nc.sync.dma_start(
            xTt0[:],
            xT_d[0:QWA, tb * 128:(tb + 1) * 128]
            .rearrange("(k p) t -> p k t", p=128))
        return inv1, xTt0

    def quantize_bounce(tb, inv1):
        # broadcast inv1 across partitions with a PE outer product
        pinv = ptp.tile([128, 128], F32, tag="pt", name="pinv")
        nc.tensor.transpose(pinv[0:1, :], inv1[:], ident_f32[:])
        invrow = small.tile([128, 128], F32, tag="invrow")
        nc.vector.tensor_copy(invrow[0:1, :], pinv[0:1, :])
        pbc = ptp.tile([128, 128], F32, tag="pt", name="pbc")
        nc.tensor.matmul(pbc[:], ones_row[0:1, :], invrow[0:1, :],
                         start=True, stop=True)
        invb = small.tile([128, 1, 128], F32, tag="invb")
        nc.vector.tensor_copy(invb[:, 0, :], pbc[:])
        invbs[tb] = invb

    def quantize_apply(tb, xTt0):
        # quantize the host-pretransposed xT directly in [h, t] layout:
        # multiply by the broadcast scale, round to int8, cast to bf16.
        NQ = 4
        QW = H // NQ
        KQ = KT1 // NQ
        invb = invbs[tb]
        xqt = xqtp.tile([128, KT1, 128], BF16, tag="xqT")
        for hh in range(NQ):
            if hh == 0:
                xTt = xTt0
            else:
                xTt = xload.tile([128, KQ, 128], F32, tag="xTt",
                                 name=f"xTt{tb}_{hh}", bufs=3)
                nc.sync.dma_start(
                    xTt[:],
                    xT_d[hh * QW:(hh + 1) * QW, tb * 128:(tb + 1) * 128]
                    .rearrange("(k p) t -> p k t", p=128))
            xq8 = xqp.tile([128, KQ, 128], I8, tag="xq_i8",
                           name=f"xq8_{hh}", bufs=3)
            nc.vector.scalar_tensor_tensor(
                xq8[:], xTt[:], 1.0, invb[:].broadcast_to([128, KQ, 128]),
                op0=ALU.mult, op1=ALU.mult)
            nc.scalar.copy(xqt[:, hh * KQ:(hh + 1) * KQ, :], xq8[:])
        xqT[tb] = xqt

    def mm1_loads(e, c0, cw):
        wg_i8 = [wload.tile([128, KT1 // 2, cw], I8, tag="wg_i8",
                            name=f"wg_i8_{e}_{c0}_{h2}", bufs=WGB)
                 for h2 in range(2)]
        wu_i8 = [wload.tile([128, KT1 // 2, cw], I8, tag="wu_i8",
                            name=f"wu_i8_{e}_{c0}_{h2}", bufs=WGB)
                 for h2 in range(2)]
        g_src = w13T_d[e, :, c0:c0 + cw].rearrange("(k p) o -> p k o", p=128)
        u_src = w13T_d[e, :, I + c0:I + c0 + cw].rearrange(
            "(k p) o -> p k o", p=128)
        for h2 in range(2):
            ksl = slice(h2 * (KT1 // 2), (h2 + 1) * (KT1 // 2))
            nc.sync.dma_start(wg_i8[h2][:], g_src[:, ksl, :])
            nc.sync.dma_start(wu_i8[h2][:], u_src[:, ksl, :])
        return wg_i8, wu_i8

    QK = QKC  # k-tiles per cast op

    def cast_quad(w_i8, kq, cw, nm):
        h2, kkq = divmod(kq, (KT1 // 2) // QK)
        ks = slice(kkq * QK, (kkq + 1) * QK)
        w_bf = wcast.tile([128, QK, cw], BF16, tag="wbf", name=nm)
        cast(w_bf[:], w_i8[h2][:, ks, :])
        return w_bf

    def mm1_epilogue(e, i_tb, tb, c0, cw, pg, pu, wsg, wsu, hts, amaxes):
        gate = outp.tile([128, cw], F32, tag="gate")
        nc.vector.scalar_tensor_tensor(
            gate[:], pg, s1s[tb][:], wsg[:, c0:c0 + cw],
            op0=ALU.mult, op1=ALU.mult)
        up = outp.tile([128, cw], F32, tag="up")
        nc.vector.scalar_tensor_tensor(
            up[:], pu, s1s[tb][:], wsu[:, c0:c0 + cw],
            op0=ALU.mult, op1=ALU.mult)
        sg = outp.tile([128, cw], F32, tag="sg")
        nc.scalar.activation(sg[:], gate[:], AF.Silu)
        nc.vector.tensor_mul(hts[i_tb][:, c0:c0 + cw], sg[:], up[:])
        # per-chunk partial abs-max keeps the requant scale off the
        # critical path (ready right after the last chunk's h lands)
        prev = amaxes[i_tb]
        amp = small.tile([128, 1], F32, tag="amax2", name=f"am2p_{i_tb}_{c0}")
        nc.vector.tensor_reduce(amp[:], hts[i_tb][:, c0:c0 + cw],
                                axis=AX.X, op=ALU.max,
                                apply_absolute_value=True)
        if prev is not None:
            amn = small.tile([128, 1], F32, tag="amax2",
                             name=f"am2_{i_tb}_{c0}")
            nc.vector.tensor_tensor(amn[:], prev[:], amp[:], op=ALU.max)
            amaxes[i_tb] = amn
        else:
            amaxes[i_tb] = amp

    def mm1_chunk(e, tbs, c0, cw, wsg, wsu, hts, amaxes, loads,
                  precast=None):
        wg_i8, wu_i8 = loads
        pg = [pgp.tile([128, cw], F32, tag="pg", name=f"pg{i}")
              for i in range(len(tbs))]
        pu = [pup.tile([128, cw], F32, tag="pu", name=f"pu{i}")
              for i in range(len(tbs))]
        for kq in range(KT1 // QK):
            if precast is not None and kq < len(precast):
                wg_bf, wu_bf = precast[kq]
            else:
                wg_bf = cast_quad(wg_i8, kq, cw, "wg_bf")
                wu_bf = cast_quad(wu_i8, kq, cw, "wu_bf")
            for dk in range(QK):
                k = kq * QK + dk
                st, sp = (k == 0), (k == KT1 - 1)
                for i_tb, tb in enumerate(tbs):
                    nc.tensor.matmul(pg[i_tb][:], xqT[tb][:, k, :],
                                     wg_bf[:, dk, :], start=st, stop=sp)
                    nc.tensor.matmul(pu[i_tb][:], xqT[tb][:, k, :],
                                     wu_bf[:, dk, :], start=st, stop=sp)
        for i_tb, tb in enumerate(tbs):
            mm1_epilogue(e, i_tb, tb, c0, cw, pg[i_tb][:], pu[i_tb][:],
                         wsg, wsu, hts, amaxes)

    def mm1_chunk_split(e, tbs, c0, cw, wsg, wsu, hts, amaxes, loads, hqT,
                        s2s):
        # last chunk: separate per-tb passes (own casts) so tb0's requant
        # chain runs under tb1's matmuls instead of stalling the PE
        wg_i8, wu_i8 = loads
        for i_tb, tb in enumerate(tbs):
            pg = pgp.tile([128, cw], F32, tag="pg", name=f"pgs{i_tb}")
            pu = pup.tile([128, cw], F32, tag="pu", name=f"pus{i_tb}")
            for kq in range(KT1 // QK):
                wg_bf = cast_quad(wg_i8, kq, cw, "wg_bf")
                wu_bf = cast_quad(wu_i8, kq, cw, "wu_bf")
                for dk in range(QK):
                    k = kq * QK + dk
                    st, sp = (k == 0), (k == KT1 - 1)
                    nc.tensor.matmul(pg[:], xqT[tb][:, k, :],
                                     wg_bf[:, dk, :], start=st, stop=sp)
                    nc.tensor.matmul(pu[:], xqT[tb][:, k, :],
                                     wu_bf[:, dk, :], start=st, stop=sp)
            mm1_epilogue(e, i_tb, tb, c0, cw, pg[:], pu[:], wsg, wsu, hts,
                         amaxes)
            hqt, s2 = requant_tb(hts[i_tb], amaxes[i_tb])
            hqT.append(hqt)
            s2s.append(s2)

    def requant_dve(ht, amax2):
        s2 = small.tile([128, 1], F32, tag="s2")
        nc.vector.tensor_scalar(s2[:], amax2[:], 1.0 / 127.0, None,
                                op0=ALU.mult)
        inv2 = small.tile([128, 1], F32, tag="inv2")
        nc.vector.reciprocal(inv2[:], s2[:])
        hq_i8 = hqp.tile([128, I], I8, tag="hq_i8")
        hq_bf = hqp.tile([128, I], BF16, tag="hq_bf")
        for a, b in ((0, 512), (512, I)):
            nc.vector.tensor_scalar(hq_i8[:, a:b], ht[:, a:b], inv2[:], None,
                                    op0=ALU.mult)
            nc.scalar.copy(hq_bf[:, a:b], hq_i8[:, a:b])
        return hq_bf, s2

    def requant_pe(hq_bf):
        hqt = hqp.tile([128, KT2, 128], BF16, tag="hqT", bufs=3)
        if HQXBAR:
            # [t, i] -> [i, t] via the DMA XBAR (ACT HWDGE ring): slower
            # per byte (~110 GB/s) but takes 11*128 cycles off the PE
            nc.scalar.dma_start_transpose(hqt[:, :, :], hq_bf[:, :])
            return hqt
        for k0 in range(0, KT2, 4):
            kn = min(4, KT2 - k0)
            pt = ptp.tile([128, 4, 128], BF16, tag="pt", name="pt_hq")
            for dk in range(kn):
                k = k0 + dk
                nc.tensor.transpose(pt[:, dk, :],
                                    hq_bf[:, k * 128:(k + 1) * 128], ident[:])
            (nc.vector.tensor_copy if (k0 // 4) % 2 else nc.scalar.copy)(
                hqt[:, k0:k0 + kn, :], pt[:, 0:kn, :])
        return hqt

    def mm2_chunk(e, tbs, c0, cw, hqT, s2s, w2s_unused=None):
        w2sc = outp.tile([128, cw], F32, tag="w2sc", bufs=4)
        nc.sync.dma_start(w2sc[:], w2s_d[e, :, c0:c0 + cw])
        w2_i8 = wload.tile([128, KT2, cw], I8, tag="w2_i8")
        nc.sync.dma_start(
            w2_i8[:],
            w2T_d[e, :, c0:c0 + cw].rearrange("(k p) o -> p k o", p=128))
        p2 = [p2p.tile([128, cw], F32, tag="p2", name=f"p2_{i}")
              for i in range(len(tbs))]
        k = 0
        for qn in (4, 4, 3):
            w2_bf = wcast.tile([128, qn, cw], BF16, tag="wbf", name="w2_bf")
            cast(w2_bf[:], w2_i8[:, k:k + qn, :])
            for dk in range(qn):
                for i_tb in range(2):
                    nc.tensor.matmul(p2[i_tb][:], hqT[i_tb][:, k, :],
                                     w2_bf[:, dk, :], start=(k == 0),
                                     stop=(k == KT2 - 1))
                k += 1
        for i_tb, tb in enumerate(tbs):
            ot = outp.tile([128, cw], F32, tag="ot", bufs=4)
            nc.vector.scalar_tensor_tensor(
                ot[:], p2[i_tb][:], s2s[i_tb][:], w2sc[:],
                op0=ALU.mult, op1=ALU.mult)
            (nc.scalar if STACT else nc.sync).dma_start(
                out_d[tb * 128:(tb + 1) * 128, c0:c0 + cw], ot[:])

    # ---- Staged two-expert pipeline ----
    # Emission order == engine-queue order, so stages are interleaved to keep
    # the PE fed across quant/requant latency chains.
    assert E_LOC == 2
    loads_ = {}
    wsgs, wsus, htss, amaxs = {}, {}, {}, {}
    hqbfs, s2ss, hqTs = {}, {}, {}

    def mm1_front(e):
        wsg = scalep.tile([128, I], F32, tag="wsg", name=f"wsg{e}")
        nc.sync.dma_start(wsg[:], wsg_d[e])
        wsu = scalep.tile([128, I], F32, tag="wsu", name=f"wsu{e}")
        nc.sync.dma_start(wsu[:], wsu_d[e])
        wsgs[e], wsus[e] = wsg, wsu
        htss[e] = [hbuf.tile([128, I], F32, tag="ht", name=f"ht{e}_{i}")
                   for i in range(2)]
        amaxs[e] = [None, None]

    def mm1_run_chunk(e, ci, precast=None):
        tbs = [2 * e, 2 * e + 1]
        c0, cw = I_CHUNKS[ci]
        ld = loads_.pop((e, ci), None)
        if ld is None:
            ld = mm1_loads(e, c0, cw)
        mm1_chunk(e, tbs, c0, cw, wsgs[e], wsus[e], htss[e], amaxs[e], ld,
                  precast=precast)

    def req_dve(e):
        hqbfs[e], s2ss[e] = [], []
        for i_tb in range(2):
            hq_bf, s2 = requant_dve(htss[e][i_tb], amaxs[e][i_tb])
            hqbfs[e].append(hq_bf)
            s2ss[e].append(s2)

    def req_pe(e):
        hqTs[e] = [requant_pe(hqbfs[e][i_tb]) for i_tb in range(2)]

    # --- expert 0 front: tb0's amax stream first, then weights ---
    if WARM and pools.get("do_warmup", True):
        warmup()
    inv_a, xt0_a = quantize_scales(0)
    loads_[(0, 0)] = mm1_loads(0, *I_CHUNKS[0])
    precast0 = [(cast_quad(loads_[(0, 0)][0], kq, I_CHUNKS[0][1], "wg_bf"),
                 cast_quad(loads_[(0, 0)][1], kq, I_CHUNKS[0][1], "wu_bf"))
                for kq in range(3)]
    inv_b, xt0_b = quantize_scales(1)
    quantize_bounce(0, inv_a)
    quantize_apply(0, xt0_a)
    quantize_bounce(1, inv_b)
    quantize_apply(1, xt0_b)
    mm1_front(0)
    mm1_run_chunk(0, 0, precast=precast0)
    # expert-1 token quant hides under expert-0 mm1
    inv_c, xt0_c = quantize_scales(2)
    inv_d, xt0_d = quantize_scales(3)
    mm1_run_chunk(0, 1)
    quantize_bounce(2, inv_c)
    quantize_bounce(3, inv_d)
    quantize_apply(2, xt0_c)
    quantize_apply(3, xt0_d)
    mm1_front(1)
    mm1_run_chunk(0, 2)
    # requant scale chain (DVE/ACT only) for e0, then e1 matmuls keep PE busy
    req_dve(0)
    mm1_run_chunk(1, 0)
    # e0 hq transposes: hq_bf has long been ready by now
    req_pe(0)
    for ci in range(5):
        mm2_chunk(0, [0, 1], *H_CHUNKS[ci], hqTs[0], s2ss[0])
    mm1_run_chunk(1, 1)
    mm1_run_chunk(1, 2)
    req_dve(1)
    # e0's mm2 tail covers e1's requant chain
    for ci in range(5, 8):
        mm2_chunk(0, [0, 1], *H_CHUNKS[ci], hqTs[0], s2ss[0])
    req_pe(1)
    for ci in range(8):
        mm2_chunk(1, [2, 3], *H_CHUNKS[ci], hqTs[1], s2ss[1])


_cached_nc = None


def _make_in_maps(x, w13, w2, w13_scale, smooth_scale_2, w2_scale):
    x = np.asarray(x, dtype=np.float32)
    w13 = np.asarray(w13).astype(np.int8, copy=False)
    w2 = np.asarray(w2).astype(np.int8, copy=False)
    w13_scale = np.asarray(w13_scale, dtype=np.float32)
    smooth_scale_2 = np.asarray(smooth_scale_2, dtype=np.float32)
    w2_scale = np.asarray(w2_scale, dtype=np.float32)

    # Fold the (linear) smooth scale into the up-projection dequant scale.
    wsu_full = w13_scale[:, I:] * smooth_scale_2          # [E, I]
    wsg_full = w13_scale[:, :I]                           # [E, I]

    in_maps = []
    for c in range(NCORES):
        es = slice(E_LOC * c, E_LOC * (c + 1))
        ts = slice(T_LOC * c, T_LOC * (c + 1))
        in_maps.append({
            "x": np.ascontiguousarray(x[ts]),
            "xT": np.ascontiguousarray(x[ts].T),
            "w13T": np.ascontiguousarray(w13[es].transpose(0, 2, 1)),
            "w2T": np.ascontiguousarray(w2[es].transpose(0, 2, 1)),
            "wsg": np.ascontiguousarray(
                np.broadcast_to(wsg_full[es, None, :], (E_LOC, 128, I))),
            "wsu": np.ascontiguousarray(
                np.broadcast_to(wsu_full[es, None, :], (E_LOC, 128, I))),
            "w2s": np.ascontiguousarray(
                np.broadcast_to(w2_scale[es, None, :], (E_LOC, 128, H))),
        })
    return in_maps


def _run(in_maps, **kwargs):
    global _cached_nc
    _install_compile_hook()
    if _cached_nc is None:
        _cached_nc = _build_program()
    return run_bass_kernel_spmd(_cached_nc, in_maps, list(range(NCORES)),
                                **kwargs)


def kernel(x, w13, w2, w13_scale, smooth_scale_2, w2_scale, expert_tokens):
    # expert_tokens describes the fixed equal contiguous grouping (the
    # reference ignores it); we rely on that same grouping.
    del expert_tokens
    in_maps = _make_in_maps(x, w13, w2, w13_scale, smooth_scale_2, w2_scale)
    res = _run(in_maps)
    return np.concatenate([res.results[c]["out"] for c in range(NCORES)],
                          axis=0)


def run_profiled(x, w13, w2, w13_scale, smooth_scale_2, w2_scale,
                 expert_tokens):
    """test.py helper: run with NTFF profiling, return BassKernelResults."""
    del expert_tokens
    in_maps = _make_in_maps(x, w13, w2, w13_scale, smooth_scale_2, w2_scale)
    return _run(in_maps, trace=True)

